# revision 1
# baseline (speedup 1.0000x reference)
"""Multi-head attention (B=2, S=2048, D=1024, H=16) on 8 TRN2 NeuronCores.

Sharding: (batch, head-group) — core c handles batch c//4 and heads
[4*(c%4), 4*(c%4)+4). Each core projects its batch's tokens onto its 4 heads'
column-shards of Wq/Wk/Wv, runs attention for those heads, and multiplies by
its row-shard of Wo, producing a partial [S, D] output. The host sums the 4
partials per batch and adds bo. No FLOP duplication across cores.

Device design notes:
  - Inputs are host-pre-transposed to feature-major X^T [D, S] so projection
    matmuls (contraction over D) stream natural, contiguous tiles.
  - Scores are computed transposed (S^T [key j, query i]) so exp(S^T) feeds
    the PV matmul directly (V as stationary operand — no transposes of the
    attention matrix). A ones column appended to V produces the softmax
    denominator in the same matmul; softmax is unshifted (scores are O(1)
    for this data, exp cannot overflow).
  - Matmul dtypes: float32r (full PE rate at N=512, ~2e-4 precision) for
    projections/QK/Wo; bf16 for exp output and V in the PV matmul.
  - Normalization: DVE reciprocal of the denominator row, SBUF->SBUF DMA hop
    to partition 0, gpsimd partition_broadcast, DVE multiply. Odd heads of a
    head-pair take a DMA hop into partitions 64-127 of the packed ctx tile so
    the output projection runs with a full K=128 contraction.
  - Emission order IS each engine's execution order (in-order streams), so
    the code emits a software-pipelined global schedule: projections are
    streamed in s-halves and attention chunks are interleaved between them;
    the j-loop is split in two psum rounds (partial evicted to SBUF) so
    attention overlaps the input-DMA ramp; the output projection for query
    half 0 is emitted between attention blocks to fill PE gaps.
  - PSUM: 2x1-bank pool for projection/transpose/Wo psums, 2x2-bank pool
    for qk score tiles, and a dedicated 1x2-bank pool for the pv accumulator
    (so the partial-eviction copy at psum-round boundaries never starves the
    qk rotation). Separate pools are required because pool slots grant in
    emission order.

Measured (8-core HW run vs fp32 reference): max-abs-err/scale = 2.1e-3.
TimelineSim cost-model estimate: ~255 us per core.
"""

import os
import numpy as np

S = 2048          # sequence length
D = 1024          # model dim
HPC = 4           # heads per core
DK = 64           # head dim
M = HPC * DK      # per-core projection width = 256
NC = 8            # cores
IW = 1024         # attention query-block width (free dim of exp / psum)

_cached = {}



def _build(debug=False):
    import concourse.bass as bass
    import concourse.bacc as bacc
    import concourse.tile as tile
    import concourse.mybir as mybir
    from contextlib import ExitStack

    f32 = mybir.dt.float32
    f32r = mybir.dt.float32r
    bf16 = mybir.dt.bfloat16
    AF = mybir.ActivationFunctionType

    def r(ap):
        return ap.bitcast(f32r)

    nc = bacc.Bacc(
        "TRN2",
        target_bir_lowering=False,
        debug=False,
        enable_asserts=False,
        num_devices=NC,
    )

    # DRAM I/O (per-core shapes)
    xqT_d = nc.dram_tensor("xqT", [D, S], f32, kind="ExternalInput").ap()
    xkT_d = nc.dram_tensor("xkT", [D, S], f32, kind="ExternalInput").ap()
    xvT_d = nc.dram_tensor("xvT", [D, S], f32, kind="ExternalInput").ap()
    wq_d = nc.dram_tensor("wq", [D, M], f32, kind="ExternalInput").ap()
    wk_d = nc.dram_tensor("wk", [D, M], f32, kind="ExternalInput").ap()
    wv_d = nc.dram_tensor("wv", [D, M], f32, kind="ExternalInput").ap()
    wo_d = nc.dram_tensor("wo", [M, D], f32, kind="ExternalInput").ap()
    bq_d = nc.dram_tensor("bq", [M], f32, kind="ExternalInput").ap()
    bk_d = nc.dram_tensor("bk", [M], f32, kind="ExternalInput").ap()
    bv_d = nc.dram_tensor("bv", [M], f32, kind="ExternalInput").ap()
    ident_d = nc.dram_tensor("ident", [128, 128], bf16, kind="ExternalInput").ap()
    out_d = nc.dram_tensor("out", [S, D], f32, kind="ExternalOutput").ap()

    NDC = D // 128     # 8 contraction chunks
    NMC = M // 128     # 2 m-chunks
    NJC = S // 128     # 16 key chunks
    NIH = S // IW      # 2 query halves
    SH = S // 2        # 1024: s-half for projection streaming

    with tile.TileContext(nc) as tc:
        with ExitStack() as outer:
            # ---- persistent pools ----
            qkv = outer.enter_context(tc.tile_pool(name="qkv", bufs=1))
            vsbp = outer.enter_context(tc.tile_pool(name="vsb", bufs=1))
            ctxp = outer.enter_context(tc.tile_pool(name="ctx", bufs=1))
            smp = outer.enter_context(tc.tile_pool(name="sm", bufs=2))
            ep = outer.enter_context(tc.tile_pool(name="ep", bufs=10))
            pcp = outer.enter_context(tc.tile_pool(name="pc", bufs=4))
            ostp = outer.enter_context(tc.tile_pool(name="ost", bufs=4))
            # PSUM pools: proj/transpose/wo via pps (2x1 bank), attn via qp (3x2 banks)
            pps = outer.enter_context(tc.tile_pool(name="pps", bufs=2, space="PSUM"))
            qp = outer.enter_context(tc.tile_pool(name="qp", bufs=2, space="PSUM"))
            pvp = outer.enter_context(tc.tile_pool(name="pvp", bufs=1, space="PSUM"))

            qT = [[qkv.tile([128, SH], f32r, tag=f"qT{m}{s}", name=f"qT{m}{s}")
                   for s in range(2)] for m in range(NMC)]
            kT = [[qkv.tile([128, SH], f32r, tag=f"kT{m}{s}", name=f"kT{m}{s}")
                  for s in range(2)] for m in range(NMC)]
            v_sb = [[vsbp.tile([128, NJC // 2, DK + 1], bf16, tag=f"v{h}{s}",
                               name=f"v{h}{s}") for s in range(2)]
                    for h in range(HPC)]
            # packed ctx^T per query-half: [dk within pair, head-pair, ih-slice]
            ctx_t = [ctxp.tile([128, NMC, IW], f32r, tag=f"ctx{i}", name=f"ctx{i}")
                     for i in range(NIH)]

            with ExitStack() as ph_a:
                wp = ph_a.enter_context(tc.tile_pool(name="wp", bufs=1))
                vtpool = ph_a.enter_context(tc.tile_pool(name="vtp", bufs=1))
                xt = ph_a.enter_context(tc.tile_pool(name="xt", bufs=9))

                wq_sb = wp.tile([128, NDC, M], f32r, tag="wq")
                wk_sb = wp.tile([128, NDC, M], f32r, tag="wk")
                wv_sb = wp.tile([128, NDC, M], f32r, tag="wv")
                bq_sb = wp.tile([128, NMC], f32, tag="bq")
                bk_sb = wp.tile([128, NMC], f32, tag="bk")
                bv_sb = wp.tile([128, NMC], f32, tag="bv")
                ident = wp.tile([128, 128], bf16, tag="ident")
                vT = [vtpool.tile([128, NMC, SH], bf16, tag=f"vT{s}", name=f"vT{s}")
                      for s in range(2)]

                w_r = lambda ap: r(ap.rearrange("(n p) m -> p n m", p=128))
                wop = ph_a.enter_context(tc.tile_pool(name="wop", bufs=1))
                wo_sb = wop.tile([128, NMC, D], f32r, tag="wo")
                _loaded = set()

                def load_w(tens):
                    if tens in _loaded:
                        return
                    _loaded.add(tens)
                    if tens == "q":
                        nc.sync.dma_start(out=wq_sb, in_=w_r(wq_d))
                        nc.sync.dma_start(
                            out=bq_sb, in_=bq_d.rearrange("(n p) -> p n", p=128)
                        )
                    elif tens == "k":
                        nc.sync.dma_start(out=wk_sb, in_=w_r(wk_d))
                        nc.sync.dma_start(
                            out=bk_sb, in_=bk_d.rearrange("(n p) -> p n", p=128)
                        )
                    else:
                        nc.sync.dma_start(out=wv_sb, in_=w_r(wv_d))
                        nc.sync.dma_start(
                            out=bv_sb, in_=bv_d.rearrange("(n p) -> p n", p=128)
                        )
                        nc.sync.dma_start(out=ident, in_=ident_d)
                for h in range(HPC):
                    for s2 in range(2):
                        nc.vector.memset(v_sb[h][s2][:, :, DK : DK + 1], 1.0)

                # ---- emission helpers (emission order IS the per-engine schedule) ----
                def emit_proj(sh):
                    s0 = sh * SH
                    for tens, xdram, w_sb, b_sb in (
                        ("q", xqT_d, wq_sb, bq_sb),
                        ("k", xkT_d, wk_sb, bk_sb),
                        ("v", xvT_d, wv_sb, bv_sb),
                    ):
                        load_w(tens)
                        xts = []
                        for dc in range(NDC):
                            t = xt.tile([128, SH], f32r, tag="x", name="x")
                            nc.sync.dma_start(
                                out=t,
                                in_=r(xdram[dc * 128 : (dc + 1) * 128, s0 : s0 + SH]),
                            )
                            xts.append(t)
                        for mc in range(NMC):
                            for sc in range(SH // 512):
                                ps = pps.tile([128, 512], f32, tag="ps", name="ps")
                                for dc in range(NDC):
                                    nc.tensor.matmul(
                                        ps,
                                        lhsT=w_sb[:, dc, mc * 128 : (mc + 1) * 128],
                                        rhs=xts[dc][:, sc * 512 : (sc + 1) * 512],
                                        start=(dc == 0),
                                        stop=(dc == NDC - 1),
                                    )
                                if tens == "q":
                                    dstap = qT[mc][sh][:, sc * 512 : (sc + 1) * 512]
                                elif tens == "k":
                                    dstap = kT[mc][sh][:, sc * 512 : (sc + 1) * 512]
                                else:
                                    dstap = vT[sh][:, mc, sc * 512 : (sc + 1) * 512]
                                nc.vector.tensor_scalar_add(
                                    out=dstap, in0=ps, scalar1=b_sb[:, mc : mc + 1]
                                )
                        if tens == "v":
                            for mc in range(NMC):
                                for sb in range(SH // 128):
                                    tp = pps.tile([128, 512], bf16, tag="ps", name="tp")
                                    nc.tensor.transpose(
                                        tp[:, 0:128],
                                        in_=vT[sh][:, mc, sb * 128 : (sb + 1) * 128],
                                        identity=ident,
                                    )
                                    nc.vector.tensor_copy(
                                        out=v_sb[2 * mc][sh][:, sb, 0:DK],
                                        in_=tp[:, 0:DK],
                                    )
                                    nc.vector.tensor_copy(
                                        out=v_sb[2 * mc + 1][sh][:, sb, 0:DK],
                                        in_=tp[:, DK:128],
                                    )

                pc0s = {}
                pvs = {}

                def emit_attn_chunk(ih, h, ksh):
                    mc, off = divmod(h, 2)
                    off *= 64
                    pv = pvp.tile([128, IW], f32, tag="pv", name="pv")
                    for kb in range(NJC // 2):
                        qk = qp.tile([128, IW], f32, tag="qp", name="qk")
                        for ha in range(IW // 512):
                            nc.tensor.matmul(
                                qk[:, ha * 512 : (ha + 1) * 512],
                                lhsT=kT[mc][ksh][
                                    off : off + DK, kb * 128 : (kb + 1) * 128
                                ],
                                rhs=qT[mc][ih][
                                    off : off + DK, ha * 512 : (ha + 1) * 512
                                ],
                                start=True,
                                stop=True,
                            )
                        e = ep.tile([128, IW], bf16, tag="e", name="e")
                        nc.scalar.activation(
                            out=e, in_=qk, func=AF.Exp, scale=1.0 / np.sqrt(DK)
                        )
                        for ha in range(IW // 512):
                            nc.tensor.matmul(
                                pv[0 : DK + 1, ha * 512 : (ha + 1) * 512],
                                lhsT=v_sb[h][ksh][:, kb, :],
                                rhs=e[:, ha * 512 : (ha + 1) * 512],
                                start=(kb == 0),
                                stop=(kb == NJC // 2 - 1),
                            )
                    if ksh == 0:
                        pc0 = pcp.tile([65, IW], f32, tag="pc", name="pc0")
                        nc.vector.tensor_copy(out=pc0, in_=pv[0:65, :])
                        pc0s[(ih, h)] = pc0
                    else:
                        pvs[(ih, h)] = pv

                def emit_normalize(ih, h):
                    mc, off = divmod(h, 2)
                    off *= 64
                    pv = pvs.pop((ih, h))
                    pc0 = pc0s.pop((ih, h))
                    s65 = smp.tile([65, IW], f32r, tag="s65", name="s65")
                    nc.vector.tensor_add(s65, pv[0:65, :], pc0)
                    inv = smp.tile([65, IW], f32, tag="inv", name="inv", bufs=1)
                    nc.vector.reciprocal(out=inv[64:65, :], in_=s65[64:65, :])
                    nc.sync.dma_start(out=inv[0:1, :], in_=inv[64:65, :])
                    bca = smp.tile([64, IW], f32, tag="bca", name="bca")
                    nc.gpsimd.partition_broadcast(bca, inv[0:1, :])
                    if off == 0:
                        nc.vector.tensor_mul(
                            ctx_t[ih][0:64, mc, :], s65[0:DK, :], bca
                        )
                    else:
                        nc.vector.tensor_mul(s65[0:DK, :], s65[0:DK, :], bca)
                        nc.sync.dma_start(
                            out=ctx_t[ih][64:128, mc, :], in_=s65[0:DK, :]
                        )

                def emit_wo(ih):
                    for icb in range(IW // 128):
                        ic = ih * (IW // 128) + icb
                        for nh in range(2):
                            ps = pps.tile([128, 512], f32, tag="ps", name="wops")
                            for g in range(NMC):
                                nc.tensor.matmul(
                                    ps,
                                    lhsT=ctx_t[ih][:, g, icb * 128 : (icb + 1) * 128],
                                    rhs=wo_sb[:, g, nh * 512 : (nh + 1) * 512],
                                    start=(g == 0),
                                    stop=(g == NMC - 1),
                                )
                            st = ostp.tile([128, 512], f32, tag="ost", name="st")
                            nc.any.tensor_copy(out=st, in_=ps)
                            nc.sync.dma_start(
                                out=out_d[
                                    ic * 128 : (ic + 1) * 128,
                                    nh * 512 : (nh + 1) * 512,
                                ],
                                in_=st,
                            )

                # ---- global interleaved schedule ----
                emit_proj(0)
                for h in range(HPC):
                    emit_attn_chunk(0, h, 0)
                emit_proj(1)
                nc.sync.dma_start(
                    out=wo_sb, in_=r(wo_d.rearrange("(g p) n -> p g n", p=128))
                )
                for h in range(HPC):
                    emit_attn_chunk(0, h, 1)
                    emit_normalize(0, h)
                for h in range(HPC):
                    emit_attn_chunk(1, h, 0)
                emit_wo(0)
                for h in range(HPC):
                    emit_attn_chunk(1, h, 1)
                    emit_normalize(1, h)
                emit_wo(1)

    nc.compile()
    return nc


def _get_nc(debug=False):
    key = ("nc", debug)
    if key not in _cached:
        _cached[key] = _build(debug)
    return _cached[key]


def _get_runner():
    """Build (once) a jitted 8-core SPMD executable mirroring
    bass2jax.run_bass_via_pjrt, reusable across calls for benchmarking."""
    if "runner" in _cached:
        return _cached["runner"]
    import jax
    import jax.numpy as jnp
    from jax.experimental.shard_map import shard_map
    from jax.sharding import Mesh, PartitionSpec
    import concourse.mybir as mybir
    from concourse import bass2jax

    bass2jax.install_neuronx_cc_hook()
    nc = _get_nc()
    assert nc.dbg_addr is None
    partition_name = nc.partition_id_tensor.name if nc.partition_id_tensor else None

    in_names, out_names, out_avals, zero_outs = [], [], [], []
    for alloc in nc.m.functions[0].allocations:
        if not isinstance(alloc, mybir.MemoryLocationSet):
            continue
        name = alloc.memorylocations[0].name
        if alloc.kind == "ExternalInput":
            if name != partition_name:
                in_names.append(name)
        elif alloc.kind == "ExternalOutput":
            out_names.append(name)
            shape = tuple(alloc.tensor_shape)
            dtype = mybir.dt.np(alloc.dtype)
            out_avals.append(jax.core.ShapedArray(shape, dtype))
            zero_outs.append(np.zeros(shape, dtype))
    n_params = len(in_names)
    all_in_names = in_names + out_names
    if partition_name is not None:
        all_in_names = all_in_names + [partition_name]
    donate = tuple(range(n_params, n_params + len(out_names)))

    def _body(*args):
        operands = list(args)
        if partition_name is not None:
            operands.append(bass2jax.partition_id_tensor())
        outs = bass2jax._bass_exec_p.bind(
            *operands,
            out_avals=tuple(out_avals),
            in_names=tuple(all_in_names),
            out_names=tuple(out_names),
            lowering_input_output_aliases=(),
            sim_require_finite=True,
            sim_require_nnan=True,
            nc=nc,
        )
        return tuple(outs)

    devices = jax.devices()[:NC]
    mesh = Mesh(np.asarray(devices), ("core",))
    nin = n_params + len(out_names)
    sharded = jax.jit(
        shard_map(
            _body,
            mesh=mesh,
            in_specs=(PartitionSpec("core"),) * nin,
            out_specs=(PartitionSpec("core"),) * len(out_names),
            check_rep=False,
        ),
        donate_argnums=donate,
        keep_unused=True,
    )

    def run(in_maps):
        concat_in = [
            np.concatenate([np.asarray(in_maps[c][n]) for c in range(NC)], axis=0)
            for n in in_names
        ]
        concat_zeros = [
            np.zeros((NC * z.shape[0], *z.shape[1:]), z.dtype) for z in zero_outs
        ]
        out_arrs = sharded(*concat_in, *concat_zeros)
        return [
            {
                n: np.asarray(out_arrs[i]).reshape(NC, *out_avals[i].shape)[c]
                for i, n in enumerate(out_names)
            }
            for c in range(NC)
        ]

    _cached["runner"] = (run, sharded, in_names, out_names, out_avals, zero_outs)
    return _cached["runner"]


def _make_in_maps(query, key, value, Wq, bq, Wk, bk, Wv, bv, Wo, bo):

    query = np.asarray(query, dtype=np.float32)
    key = np.asarray(key, dtype=np.float32)
    value = np.asarray(value, dtype=np.float32)
    Wq, Wk, Wv, Wo = (np.asarray(a, dtype=np.float32) for a in (Wq, Wk, Wv, Wo))
    bq, bk, bv, bo = (np.asarray(a, dtype=np.float32) for a in (bq, bk, bv, bo))
    B = query.shape[0]
    import ml_dtypes
    ident = np.eye(128, dtype=ml_dtypes.bfloat16)

    xqT = [np.ascontiguousarray(query[b].T) for b in range(B)]
    xkT = [np.ascontiguousarray(key[b].T) for b in range(B)]
    xvT = [np.ascontiguousarray(value[b].T) for b in range(B)]

    in_maps = []
    for c in range(NC):
        b, hg = divmod(c, NC // B)
        sl = slice(hg * M, (hg + 1) * M)
        in_maps.append(
            {
                "xqT": xqT[b],
                "xkT": xkT[b],
                "xvT": xvT[b],
                "wq": np.ascontiguousarray(Wq[:, sl]),
                "wk": np.ascontiguousarray(Wk[:, sl]),
                "wv": np.ascontiguousarray(Wv[:, sl]),
                "wo": np.ascontiguousarray(Wo[sl, :]),
                "bq": np.ascontiguousarray(bq[sl]),
                "bk": np.ascontiguousarray(bk[sl]),
                "bv": np.ascontiguousarray(bv[sl]),
                "ident": ident,
            }
        )
    return in_maps


def kernel(query, key, value, Wq, bq, Wk, bk, Wv, bv, Wo, bo):
    in_maps = _make_in_maps(query, key, value, Wq, bq, Wk, bk, Wv, bv, Wo, bo)
    run = _get_runner()[0]
    results = run(in_maps)

    B = np.asarray(query).shape[0]
    bo = np.asarray(bo, dtype=np.float32)
    full = np.zeros((B, S, D), np.float32)
    for b in range(B):
        acc = np.zeros((S, D), np.float32)
        for g in range(NC // B):
            acc += results[b * (NC // B) + g]["out"]
        full[b] = acc + bo[None, :]
    return full



# revision 4
# speedup vs baseline: 1.0365x; 1.0365x over previous
"""Multi-head attention (B=2, S=2048, D=1024, H=16) on 8 TRN2 NeuronCores.

Sharding: (batch, head-group) — core c handles batch c//4 and heads
[4*(c%4), 4*(c%4)+4). Each core projects its batch's tokens onto its 4 heads'
column-shards of Wq/Wk/Wv, runs attention for those heads, and multiplies by
its row-shard of Wo, producing a partial [S, D] output. The host sums the 4
partials per batch and adds (bo + bv @ Wo). bk is dropped entirely (a key
bias shifts every score of a query by the same constant, which softmax
cancels); bv contributes bv @ Wo to the output because attention weights sum
to one.

Device design notes:
  - All matmuls run in bf16 (1 cycle/row at any free size on the PE cost
    model); inputs and weights are cast to bf16 on the host, halving input
    DMA. PSUM accumulation stays f32.
  - Q/K are projected feature-major (out [m, s]; W stationary). V is
    projected token-major (x stationary, Wv moving) giving v in [keys, dk]
    layout directly — no V transpose pass. A constant-1 column appended to
    each per-(key-chunk, head) V block produces the softmax denominator
    inside the PV matmul.
  - Scores are computed transposed (S^T [key, query]) in [128, 2048] PSUM
    tiles (4 banks) so each Exp activation covers 2048 elements/partition,
    minimizing Act-engine fixed overhead. Act is the #2 engine (~121us).
  - PV is computed with queries on the OUTPUT partitions: out[q, dk+1] +=
    e_chunk^T-slice @ v_chunk. Free dim is 65 instead of 512, so PV costs
    half the baseline's PE cycles. Softmax normalization becomes a
    per-partition scalar multiply (DVE reciprocal of the denominator column
    + tensor_scalar_mul) — no partition broadcast needed.
  - ctx [q, m] is then PE-transposed per 128x128 block into ctx^T [m, q]
    for the Wo projection (contraction over m).
  - Emission order IS each engine's execution order. The schedule runs 8
    attention units (ih half x head) paced by the Act engine's exp stream;
    projections, PV of earlier units, transposes and Wo chunks are
    interleaved between score tiles via a static filler table.
  - PSUM: 2x1-bank pool (projections/transposes/Wo), 1x4-bank score tile,
    2x1-bank PV accumulators ([128, 4, 65] f32). Total exactly 8 banks.
"""

import numpy as np

S = 2048          # sequence length
D = 1024          # model dim
HPC = 4           # heads per core
DK = 64           # head dim
M = HPC * DK      # per-core projection width = 256
NC = 8            # cores
IW = 1024         # attention query width per ih-half
NDC = D // 128    # 8 contraction chunks
NMC = M // 128    # 2 m-chunks (head pairs)
NKB = S // 128    # 16 key chunks
EXPW = 2048       # exp tile width (2 key chunks per tile)
NT = 16 * IW // EXPW  # qk/exp tiles per unit = 8

_cached = {}


def _build(debug=False):
    import concourse.bass as bass
    import concourse.bacc as bacc
    import concourse.tile as tile
    import concourse.mybir as mybir
    from contextlib import ExitStack

    f32 = mybir.dt.float32
    bf16 = mybir.dt.bfloat16
    AF = mybir.ActivationFunctionType

    nc = bacc.Bacc(
        "TRN2",
        target_bir_lowering=False,
        debug=False,
        enable_asserts=False,
        num_devices=NC,
    )

    # DRAM I/O (per-core shapes)
    xqT_d = nc.dram_tensor("xqT", [D, S], bf16, kind="ExternalInput").ap()
    xkT_d = nc.dram_tensor("xkT", [D, S], bf16, kind="ExternalInput").ap()
    xvT_d = nc.dram_tensor("xvT", [D, S], bf16, kind="ExternalInput").ap()
    wq_d = nc.dram_tensor("wq", [D, M], bf16, kind="ExternalInput").ap()
    wk_d = nc.dram_tensor("wk", [D, M], bf16, kind="ExternalInput").ap()
    wv_d = nc.dram_tensor("wv", [D, M], bf16, kind="ExternalInput").ap()
    wo_d = nc.dram_tensor("wo", [M, D], bf16, kind="ExternalInput").ap()
    bq_d = nc.dram_tensor("bq", [M], f32, kind="ExternalInput").ap()
    ident_d = nc.dram_tensor("ident", [128, 128], bf16, kind="ExternalInput").ap()
    out_d = nc.dram_tensor("out", [S, D], f32, kind="ExternalOutput").ap()

    with tile.TileContext(nc) as tc:
        with ExitStack() as st:
            wp = st.enter_context(tc.tile_pool(name="wp", bufs=1))
            xt = st.enter_context(tc.tile_pool(name="xt", bufs=18))
            qkt = st.enter_context(tc.tile_pool(name="qkt", bufs=1))
            vp = st.enter_context(tc.tile_pool(name="vp", bufs=1))
            ep = st.enter_context(tc.tile_pool(name="ep", bufs=24))
            stp = st.enter_context(tc.tile_pool(name="stp", bufs=1))
            ctp = st.enter_context(tc.tile_pool(name="ctp", bufs=1))
            invp = st.enter_context(tc.tile_pool(name="invp", bufs=4))
            ostp = st.enter_context(tc.tile_pool(name="ostp", bufs=4))
            # PSUM: exactly 8 banks
            psp = st.enter_context(tc.tile_pool(name="psp", bufs=2, space="PSUM"))
            qkp = st.enter_context(tc.tile_pool(name="qkp", bufs=1, space="PSUM"))
            pvp = st.enter_context(tc.tile_pool(name="pvp", bufs=2, space="PSUM"))

            wq_sb = wp.tile([128, NDC, M], bf16, tag="wq")
            wk_sb = wp.tile([128, NDC, M], bf16, tag="wk")
            wv_sb = wp.tile([128, NDC, M], bf16, tag="wv")
            wo_sb = wp.tile([128, NMC, D], bf16, tag="wo")
            bq_sb = wp.tile([128, NMC], f32, tag="bq")
            ident = wp.tile([128, 128], bf16, tag="ident")
            qT = [qkt.tile([128, S], bf16, tag=f"qT{m}", name=f"qT{m}")
                  for m in range(NMC)]
            kT = [qkt.tile([128, S], bf16, tag=f"kT{m}", name=f"kT{m}")
                  for m in range(NMC)]
            # v in [keys, head, dk+1] layout; col DK is the constant 1
            v_sb = vp.tile([128, NKB, HPC, DK + 1], bf16, tag="v")
            stage = [[stp.tile([128, M], bf16, tag=f"st{i}{q}", name=f"st{i}{q}")
                      for q in range(8)] for i in range(2)]
            ctx_t = [[ctp.tile([128, IW], bf16, tag=f"ct{i}{m}", name=f"ct{i}{m}")
                      for m in range(NMC)] for i in range(2)]

            nc.vector.memset(v_sb[:, :, :, DK:DK + 1], 1.0)

            w_r = lambda ap: ap.rearrange("(n p) m -> p n m", p=128)

            # ---- input DMA emission (order = SP.SEQ issue order) ----
            nc.sync.dma_start(out=bq_sb, in_=bq_d.rearrange("(n p) -> p n", p=128))
            nc.sync.dma_start(out=ident, in_=ident_d)
            nc.sync.dma_start(out=wk_sb, in_=w_r(wk_d))
            nc.sync.dma_start(out=wq_sb, in_=w_r(wq_d))
            xtiles = {}  # (tensor, dc, sh) -> tile

            def load_x(tens, dram, sh):
                for dc in range(NDC):
                    t = xt.tile([128, 1024], bf16, tag="x", name=f"x{tens}{dc}{sh}")
                    nc.sync.dma_start(
                        out=t, in_=dram[dc * 128:(dc + 1) * 128,
                                        sh * 1024:(sh + 1) * 1024])
                    xtiles[(tens, dc, sh)] = t

            load_x("k", xkT_d, 0)
            load_x("q", xqT_d, 0)
            load_x("k", xkT_d, 1)
            nc.sync.dma_start(out=wv_sb, in_=w_r(wv_d))
            load_x("v", xvT_d, 0)
            load_x("v", xvT_d, 1)
            load_x("q", xqT_d, 1)
            nc.sync.dma_start(out=wo_sb, in_=wo_d.rearrange("(g p) n -> p g n", p=128))

            # ---- emission helpers ----
            def proj_mk(tens, sh, mc, sc):
                """Project q/k chunk: out [m 128, s 512]; W stationary."""
                w_sb = wq_sb if tens == "q" else wk_sb
                ps = psp.tile([128, 512], f32, tag="ps", name="ps")
                for dc in range(NDC):
                    nc.tensor.matmul(
                        ps,
                        lhsT=w_sb[:, dc, mc * 128:(mc + 1) * 128],
                        rhs=xtiles[(tens, dc, sh)][:, sc * 512:(sc + 1) * 512],
                        start=(dc == 0), stop=(dc == NDC - 1))
                dst = (qT if tens == "q" else kT)[mc][
                    :, sh * 1024 + sc * 512: sh * 1024 + (sc + 1) * 512]
                if tens == "q":
                    nc.vector.tensor_scalar_add(
                        out=dst, in0=ps, scalar1=bq_sb[:, mc:mc + 1])
                else:
                    nc.vector.tensor_copy(out=dst, in_=ps)

            def proj_v(kb):
                """Project v key-chunk kb: out [s 128, m 256]; x stationary."""
                sh, sc = divmod(kb, 8)
                ps = psp.tile([128, 512], f32, tag="ps", name="ps")
                for dc in range(NDC):
                    nc.tensor.matmul(
                        ps[:, 0:M],
                        lhsT=xtiles[("v", dc, sh)][:, sc * 128:(sc + 1) * 128],
                        rhs=wv_sb[:, dc, :],
                        start=(dc == 0), stop=(dc == NDC - 1))
                for h in range(HPC):
                    nc.vector.tensor_copy(
                        out=v_sb[:, kb, h, 0:DK],
                        in_=ps[:, h * DK:(h + 1) * DK])

            e_tiles = {}

            def qk_tile(u, t):
                """Scores^T for kb pair (2t, 2t+1) x queries of ih: exp -> e."""
                ih, h = divmod(u, HPC)
                mc, off = divmod(h, 2)
                off *= DK
                qk = qkp.tile([128, EXPW], f32, tag="qk", name="qk")
                for j in range(EXPW // IW):
                    kb = t * (EXPW // IW) + j
                    for ha in range(IW // 512):
                        nc.tensor.matmul(
                            qk[:, j * IW + ha * 512: j * IW + (ha + 1) * 512],
                            lhsT=kT[mc][off:off + DK, kb * 128:(kb + 1) * 128],
                            rhs=qT[mc][off:off + DK,
                                       ih * IW + ha * 512: ih * IW + (ha + 1) * 512],
                            start=True, stop=True)
                e = ep.tile([128, EXPW], bf16, tag="e", name=f"e{u}_{t}")
                nc.scalar.activation(out=e, in_=qk, func=AF.Exp,
                                     scale=1.0 / np.sqrt(DK))
                e_tiles[(u, t)] = e

            pv_psum = {}

            def pv_qc(u, qc):
                """ctx[q 128, dk+1] for queries qc of unit u; accumulate all kb.
                Then normalize into stage (per-partition scalar multiply)."""
                ih, h = divmod(u, HPC)
                qg, q4 = divmod(qc, 4)
                if q4 == 0:
                    pv_psum[(u, qg)] = pvp.tile([128, 4, DK + 1], f32,
                                                tag="pv", name="pv")
                pv = pv_psum[(u, qg)]
                for kb in range(NKB):
                    t, j = divmod(kb, EXPW // IW)
                    e = e_tiles[(u, t)]
                    nc.tensor.matmul(
                        pv[:, q4, :],
                        lhsT=e[:, j * IW + qc * 128: j * IW + (qc + 1) * 128],
                        rhs=v_sb[:, kb, h, :],
                        start=(kb == 0), stop=(kb == NKB - 1))
                inv = invp.tile([128, 1], f32, tag="inv", name="inv")
                nc.vector.reciprocal(out=inv, in_=pv[:, q4, DK:DK + 1])
                nc.vector.tensor_scalar_mul(
                    out=stage[ih][qc][:, h * DK:(h + 1) * DK],
                    in0=pv[:, q4, 0:DK], scalar1=inv)

            def t_qc(ih, qc, mc):
                """Transpose normalized ctx block [q 128, m 128] -> ctx_t."""
                ps = psp.tile([128, 512], bf16, tag="ps", name="tp")
                nc.tensor.transpose(
                    ps[:, 0:128],
                    in_=stage[ih][qc][:, mc * 128:(mc + 1) * 128],
                    identity=ident)
                nc.vector.tensor_copy(
                    out=ctx_t[ih][mc][:, qc * 128:(qc + 1) * 128],
                    in_=ps[:, 0:128])

            def wo_qc(ih, qc):
                """Output projection for query chunk qc of half ih."""
                for nh in range(2):
                    ps = psp.tile([128, 512], f32, tag="ps", name="wops")
                    for mc in range(NMC):
                        nc.tensor.matmul(
                            ps,
                            lhsT=ctx_t[ih][mc][:, qc * 128:(qc + 1) * 128],
                            rhs=wo_sb[:, mc, nh * 512:(nh + 1) * 512],
                            start=(mc == 0), stop=(mc == NMC - 1))
                    os_ = ostp.tile([128, 512], f32, tag="ost", name="ost")
                    nc.vector.tensor_copy(out=os_, in_=ps)
                    nc.sync.dma_start(
                        out=out_d[(ih * 8 + qc) * 128:(ih * 8 + qc + 1) * 128,
                                  nh * 512:(nh + 1) * 512],
                        in_=os_)

            # ---- static filler schedule ----
            K1 = [lambda mc=mc, sc=sc: proj_mk("k", 1, mc, sc)
                  for mc in range(2) for sc in range(2)]
            Q1 = [lambda mc=mc, sc=sc: proj_mk("q", 1, mc, sc)
                  for mc in range(2) for sc in range(2)]
            V0 = [lambda kb=kb: proj_v(kb) for kb in range(8)]
            V1 = [lambda kb=kb: proj_v(kb) for kb in range(8, 16)]
            PV = lambda u, qc: (lambda: pv_qc(u, qc))
            T = lambda ih, mc: (lambda: [t_qc(ih, qc, mc) for qc in range(8)])
            WO = lambda ih, qc: (lambda: wo_qc(ih, qc))

            FILL = {
                (0, 2): [K1[0]], (0, 3): [K1[1]], (0, 5): [K1[2]], (0, 6): [K1[3]],
                (1, 0): [V0[0]], (1, 1): [V0[1]], (1, 2): [V0[2]], (1, 3): [V0[3]],
                (1, 4): [V0[4]], (1, 5): [V0[5]], (1, 6): [V0[6]], (1, 7): [V0[7]],
                (2, 0): [V1[0], V1[1]], (2, 1): [V1[2], V1[3]],
                (2, 2): [V1[4], V1[5]], (2, 3): [V1[6], V1[7]],
                (2, 4): [PV(0, 0), PV(0, 1)], (2, 5): [PV(0, 2), PV(0, 3)],
                (2, 6): [PV(0, 4), PV(0, 5)], (2, 7): [PV(0, 6), PV(0, 7)],
                (3, 0): [Q1[0]], (3, 1): [Q1[1]], (3, 2): [Q1[2]], (3, 3): [Q1[3]],
                (3, 4): [PV(1, 0), PV(1, 1)], (3, 5): [PV(1, 2), PV(1, 3)],
                (3, 6): [PV(1, 4), PV(1, 5)], (3, 7): [PV(1, 6), PV(1, 7)],
                (4, 0): [T(0, 0)],
                (4, 1): [PV(2, 0), PV(2, 1)], (4, 2): [PV(2, 2), PV(2, 3)],
                (4, 3): [PV(2, 4), PV(2, 5)], (4, 4): [PV(2, 6), PV(2, 7)],
                (4, 5): [PV(3, 0), PV(3, 1)], (4, 6): [PV(3, 2), PV(3, 3)],
                (4, 7): [PV(3, 4), PV(3, 5)],
                (5, 0): [PV(3, 6), PV(3, 7)], (5, 1): [T(0, 1)],
                (5, 2): [WO(0, 0)], (5, 3): [WO(0, 1)], (5, 4): [WO(0, 2)],
                (5, 5): [WO(0, 3)], (5, 6): [WO(0, 4)], (5, 7): [WO(0, 5)],
                (6, 0): [WO(0, 6)], (6, 1): [WO(0, 7)],
                (6, 2): [PV(4, 0), PV(4, 1)], (6, 3): [PV(4, 2), PV(4, 3)],
                (6, 4): [PV(4, 4), PV(4, 5)], (6, 5): [PV(4, 6), PV(4, 7)],
                (6, 6): [PV(5, 0), PV(5, 1)], (6, 7): [PV(5, 2), PV(5, 3)],
                (7, 0): [PV(5, 4), PV(5, 5)], (7, 1): [PV(5, 6), PV(5, 7)],
                (7, 2): [T(1, 0)],
                (7, 3): [PV(6, 0), PV(6, 1)], (7, 4): [PV(6, 2), PV(6, 3)],
                (7, 5): [PV(6, 4), PV(6, 5)], (7, 6): [PV(6, 6), PV(6, 7)],
            }

            # ---- main pipeline ----
            # prologue: K and Q projections for the first halves
            for mc in range(2):
                for sc in range(2):
                    proj_mk("k", 0, mc, sc)
            for mc in range(2):
                for sc in range(2):
                    proj_mk("q", 0, mc, sc)
            # 8 attention units paced by the exp stream
            for u in range(8):
                for t in range(NT):
                    qk_tile(u, t)
                    for fn in FILL.get((u, t), []):
                        fn()
            # tail: close out the last unit and the second half's Wo
            for qc in range(8):
                pv_qc(7, qc)
            for qc in range(8):
                t_qc(1, qc, 1)
                wo_qc(1, qc)

    nc.compile()
    return nc


def _get_nc(debug=False):
    key = ("nc", debug)
    if key not in _cached:
        _cached[key] = _build(debug)
    return _cached[key]


def _get_runner():
    """Build (once) a jitted 8-core SPMD executable mirroring
    bass2jax.run_bass_via_pjrt, reusable across calls for benchmarking."""
    if "runner" in _cached:
        return _cached["runner"]
    import jax
    import jax.numpy as jnp
    from jax.experimental.shard_map import shard_map
    from jax.sharding import Mesh, PartitionSpec
    import concourse.mybir as mybir
    from concourse import bass2jax

    bass2jax.install_neuronx_cc_hook()
    nc = _get_nc()
    assert nc.dbg_addr is None
    partition_name = nc.partition_id_tensor.name if nc.partition_id_tensor else None

    in_names, out_names, out_avals, zero_outs = [], [], [], []
    for alloc in nc.m.functions[0].allocations:
        if not isinstance(alloc, mybir.MemoryLocationSet):
            continue
        name = alloc.memorylocations[0].name
        if alloc.kind == "ExternalInput":
            if name != partition_name:
                in_names.append(name)
        elif alloc.kind == "ExternalOutput":
            out_names.append(name)
            shape = tuple(alloc.tensor_shape)
            dtype = mybir.dt.np(alloc.dtype)
            out_avals.append(jax.core.ShapedArray(shape, dtype))
            zero_outs.append(np.zeros(shape, dtype))
    n_params = len(in_names)
    all_in_names = in_names + out_names
    if partition_name is not None:
        all_in_names = all_in_names + [partition_name]
    donate = tuple(range(n_params, n_params + len(out_names)))

    def _body(*args):
        operands = list(args)
        if partition_name is not None:
            operands.append(bass2jax.partition_id_tensor())
        outs = bass2jax._bass_exec_p.bind(
            *operands,
            out_avals=tuple(out_avals),
            in_names=tuple(all_in_names),
            out_names=tuple(out_names),
            lowering_input_output_aliases=(),
            sim_require_finite=True,
            sim_require_nnan=True,
            nc=nc,
        )
        return tuple(outs)

    devices = jax.devices()[:NC]
    mesh = Mesh(np.asarray(devices), ("core",))
    nin = n_params + len(out_names)
    sharded = jax.jit(
        shard_map(
            _body,
            mesh=mesh,
            in_specs=(PartitionSpec("core"),) * nin,
            out_specs=(PartitionSpec("core"),) * len(out_names),
            check_rep=False,
        ),
        donate_argnums=donate,
        keep_unused=True,
    )

    def run(in_maps):
        concat_in = [
            np.concatenate([np.asarray(in_maps[c][n]) for c in range(NC)], axis=0)
            for n in in_names
        ]
        concat_zeros = [
            np.zeros((NC * z.shape[0], *z.shape[1:]), z.dtype) for z in zero_outs
        ]
        out_arrs = sharded(*concat_in, *concat_zeros)
        return [
            {
                n: np.asarray(out_arrs[i]).reshape(NC, *out_avals[i].shape)[c]
                for i, n in enumerate(out_names)
            }
            for c in range(NC)
        ]

    _cached["runner"] = (run, sharded, in_names, out_names, out_avals, zero_outs)
    return _cached["runner"]


def _make_in_maps(query, key, value, Wq, bq, Wk, bk, Wv, bv, Wo, bo):
    import ml_dtypes
    bf16 = ml_dtypes.bfloat16

    query = np.asarray(query, dtype=np.float32)
    key = np.asarray(key, dtype=np.float32)
    value = np.asarray(value, dtype=np.float32)
    Wq, Wk, Wv, Wo = (np.asarray(a, dtype=np.float32) for a in (Wq, Wk, Wv, Wo))
    bq = np.asarray(bq, dtype=np.float32)
    B = query.shape[0]
    ident = np.eye(128, dtype=bf16)

    xqT = [np.ascontiguousarray(query[b].T).astype(bf16) for b in range(B)]
    xkT = [np.ascontiguousarray(key[b].T).astype(bf16) for b in range(B)]
    xvT = [np.ascontiguousarray(value[b].T).astype(bf16) for b in range(B)]

    in_maps = []
    for c in range(NC):
        b, hg = divmod(c, NC // B)
        sl = slice(hg * M, (hg + 1) * M)
        in_maps.append(
            {
                "xqT": xqT[b],
                "xkT": xkT[b],
                "xvT": xvT[b],
                "wq": np.ascontiguousarray(Wq[:, sl]).astype(bf16),
                "wk": np.ascontiguousarray(Wk[:, sl]).astype(bf16),
                "wv": np.ascontiguousarray(Wv[:, sl]).astype(bf16),
                "wo": np.ascontiguousarray(Wo[sl, :]).astype(bf16),
                "bq": np.ascontiguousarray(bq[sl]),
                "ident": ident,
            }
        )
    return in_maps


def kernel(query, key, value, Wq, bq, Wk, bk, Wv, bv, Wo, bo):
    in_maps = _make_in_maps(query, key, value, Wq, bq, Wk, bk, Wv, bv, Wo, bo)
    run = _get_runner()[0]
    results = run(in_maps)

    B = np.asarray(query).shape[0]
    bo = np.asarray(bo, dtype=np.float32)
    bv = np.asarray(bv, dtype=np.float32)
    Wo_f = np.asarray(Wo, dtype=np.float32)
    base = bo + bv @ Wo_f  # bv contributes exactly bv @ Wo (sum of attn = 1)
    full = np.zeros((B, S, D), np.float32)
    for b in range(B):
        acc = np.zeros((S, D), np.float32)
        for g in range(NC // B):
            acc += results[b * (NC // B) + g]["out"]
        full[b] = acc + base[None, :]
    return full


# revision 10
# speedup vs baseline: 1.3991x; 1.3498x over previous
"""Multi-head attention (B=2, S=2048, D=1024, H=16) on 8 TRN2 NeuronCores.

Sharding: (batch, head-group) — core c handles batch c//4 and heads
[4*(c%4), 4*(c%4)+4). Each core projects its batch's tokens onto its 4 heads'
column-shards of Wq/Wk/Wv, runs attention for those heads, and multiplies by
its row-shard of Wo, producing a partial [S, D] output. The host sums the 4
partials per batch and adds (bo + bv @ Wo). bk is dropped entirely (a key
bias shifts every score of a query by the same constant, which softmax
cancels); bv contributes bv @ Wo to the output because attention weights sum
to one.

Device design notes:
  - All matmuls run in bf16 (1 cycle/row at any free size on the PE cost
    model); inputs and weights are cast to bf16 on the host, halving input
    DMA. PSUM accumulation stays f32.
  - Q/K are projected feature-major (out [m, s]; W stationary). V is
    projected token-major (x stationary, Wv moving) giving v in [keys, dk]
    layout directly — no V transpose pass. A constant-1 column appended to
    each per-(key-chunk, head) V block produces the softmax denominator
    inside the PV matmul.
  - Scores are computed transposed (S^T [key, query]) in [128, 2048] PSUM
    tiles (4 banks) so each Exp activation covers 2048 elements/partition,
    minimizing Act-engine fixed overhead. Act is the #2 engine (~121us).
  - PV is computed with queries on the OUTPUT partitions: out[q, dk+1] +=
    e_chunk^T-slice @ v_chunk. Free dim is 65 instead of 512, so PV costs
    half the baseline's PE cycles. Softmax normalization becomes a
    per-partition scalar multiply (DVE reciprocal of the denominator column
    + tensor_scalar_mul) — no partition broadcast needed.
  - ctx [q, m] is then PE-transposed per 128x128 block into ctx^T [m, q]
    for the Wo projection (contraction over m).
  - Emission order IS each engine's execution order. The schedule runs 8
    attention units (ih half x head) paced by the Act engine's exp stream;
    projections, PV of earlier units, transposes and Wo chunks are
    interleaved between score tiles via a static filler table.
  - PSUM: 2x1-bank pool (projections/transposes/Wo), 1x4-bank score tile,
    2x1-bank PV accumulators ([128, 4, 65] f32). Total exactly 8 banks.
"""

import numpy as np

S = 2048          # sequence length
D = 1024          # model dim
HPC = 4           # heads per core
DK = 64           # head dim
M = HPC * DK      # per-core projection width = 256
NC = 8            # cores
IW = 1024         # attention query width per ih-half
NDC = D // 128    # 8 contraction chunks
NMC = M // 128    # 2 m-chunks (head pairs)
NKB = S // 128    # 16 key chunks
EXPW = 1024       # exp tile width
NT = 16 * IW // EXPW  # qk/exp tiles per unit = 16

_cached = {}


def _build(debug=False):
    import concourse.bass as bass
    import concourse.bacc as bacc
    import concourse.tile as tile
    import concourse.mybir as mybir
    from contextlib import ExitStack

    f32 = mybir.dt.float32
    bf16 = mybir.dt.bfloat16
    AF = mybir.ActivationFunctionType

    nc = bacc.Bacc(
        "TRN2",
        target_bir_lowering=False,
        debug=False,
        enable_asserts=False,
        num_devices=NC,
    )

    # DRAM I/O (per-core shapes)
    xqT_d = nc.dram_tensor("xqT", [D, S], bf16, kind="ExternalInput").ap()
    xkT_d = nc.dram_tensor("xkT", [D, S], bf16, kind="ExternalInput").ap()
    xvT_d = nc.dram_tensor("xvT", [D, S], bf16, kind="ExternalInput").ap()
    wq_d = nc.dram_tensor("wq", [D, M], bf16, kind="ExternalInput").ap()
    wk_d = nc.dram_tensor("wk", [D, M], bf16, kind="ExternalInput").ap()
    wv_d = nc.dram_tensor("wv", [D, M], bf16, kind="ExternalInput").ap()
    wo_d = nc.dram_tensor("wo", [M, D], bf16, kind="ExternalInput").ap()
    bq_d = nc.dram_tensor("bq", [M], f32, kind="ExternalInput").ap()
    ident_d = nc.dram_tensor("ident", [128, 128], bf16, kind="ExternalInput").ap()
    out_d = nc.dram_tensor("out", [S, D], f32, kind="ExternalOutput").ap()

    with tile.TileContext(nc) as tc:
        with ExitStack() as st:
            wp = st.enter_context(tc.tile_pool(name="wp", bufs=1))
            xt = st.enter_context(tc.tile_pool(name="xt", bufs=16))
            xsp = st.enter_context(tc.tile_pool(name="xsp", bufs=16))
            qkt = st.enter_context(tc.tile_pool(name="qkt", bufs=1))
            vp = st.enter_context(tc.tile_pool(name="vp", bufs=1))
            ep = st.enter_context(tc.tile_pool(name="ep", bufs=40))
            stp = st.enter_context(tc.tile_pool(name="stp", bufs=1))
            ctp = st.enter_context(tc.tile_pool(name="ctp", bufs=1))
            invp = st.enter_context(tc.tile_pool(name="invp", bufs=4))
            ostp = st.enter_context(tc.tile_pool(name="ostp", bufs=4))
            # PSUM: exactly 8 banks
            psp = st.enter_context(tc.tile_pool(name="psp", bufs=2, space="PSUM"))
            qkp = st.enter_context(tc.tile_pool(name="qkp", bufs=2, space="PSUM"))
            pvp = st.enter_context(tc.tile_pool(name="pvp", bufs=2, space="PSUM"))

            wq_sb = wp.tile([128, NDC, M], bf16, tag="wq")
            wk_sb = wp.tile([128, NDC, M], bf16, tag="wk")
            wv_sb = wp.tile([128, NDC, M], bf16, tag="wv")
            wo_sb = wp.tile([128, NMC, D], bf16, tag="wo")
            bq_sb = wp.tile([128, NMC], f32, tag="bq")
            ident = wp.tile([128, 128], bf16, tag="ident")
            qT = [qkt.tile([128, S], bf16, tag=f"qT{m}", name=f"qT{m}")
                  for m in range(NMC)]
            kT = [qkt.tile([128, S], bf16, tag=f"kT{m}", name=f"kT{m}")
                  for m in range(NMC)]
            # v in [keys, head, dk+1] layout; col DK is the constant 1
            v_sb = vp.tile([128, NKB, HPC, DK + 1], bf16, tag="v")
            stage = [[stp.tile([128, M], bf16, tag=f"st{i}{q}", name=f"st{i}{q}")
                      for q in range(8)] for i in range(2)]
            ctx_t = [[ctp.tile([128, IW], bf16, tag=f"ct{i}{m}", name=f"ct{i}{m}")
                      for m in range(NMC)] for i in range(2)]

            nc.vector.memset(v_sb[:, :, :, DK:DK + 1], 1.0)

            w_r = lambda ap: ap.rearrange("(n p) m -> p n m", p=128)

            # ---- input DMA emission (order = SP.SEQ issue order) ----
            nc.sync.dma_start(out=bq_sb, in_=bq_d.rearrange("(n p) -> p n", p=128))
            nc.sync.dma_start(out=ident, in_=ident_d)
            xtiles = {}   # (tensor, dc, sh) -> [128, 1024] tile
            xstiles = {}  # ("k", dc, sc) -> [128, 512] tile (sh0, sc-granular)

            def load_x(tens, dram, sh):
                for dc in range(NDC):
                    t = xt.tile([128, 1024], bf16, tag="x", name=f"x{tens}{dc}{sh}")
                    nc.sync.dma_start(
                        out=t, in_=dram[dc * 128:(dc + 1) * 128,
                                        sh * 1024:(sh + 1) * 1024])
                    xtiles[(tens, dc, sh)] = t

            def load_xk0(sc):
                for dc in range(NDC):
                    t = xsp.tile([128, 512], bf16, tag="xs", name=f"xk{dc}s{sc}")
                    nc.sync.dma_start(
                        out=t, in_=xkT_d[dc * 128:(dc + 1) * 128,
                                         sc * 512:(sc + 1) * 512])
                    xstiles[("k", dc, sc)] = t

            nc.sync.dma_start(out=wk_sb, in_=w_r(wk_d))
            load_xk0(0)
            nc.sync.dma_start(out=wq_sb, in_=w_r(wq_d))
            load_x("q", xqT_d, 0)
            load_xk0(1)
            load_x("k", xkT_d, 1)
            nc.sync.dma_start(out=wv_sb, in_=w_r(wv_d))
            load_x("v", xvT_d, 0)
            load_x("v", xvT_d, 1)
            load_x("q", xqT_d, 1)
            nc.sync.dma_start(out=wo_sb, in_=wo_d.rearrange("(g p) n -> p g n", p=128))

            # ---- emission helpers ----
            def proj_mk(tens, sh, mc, sc):
                """Project q/k chunk: out [m 128, s 512]; W stationary."""
                w_sb = wq_sb if tens == "q" else wk_sb
                ps = psp.tile([128, 512], f32, tag="ps", name="ps")
                for dc in range(NDC):
                    if tens == "k" and sh == 0:
                        rhs = xstiles[("k", dc, sc)]
                    else:
                        rhs = xtiles[(tens, dc, sh)][:, sc * 512:(sc + 1) * 512]
                    nc.tensor.matmul(
                        ps,
                        lhsT=w_sb[:, dc, mc * 128:(mc + 1) * 128],
                        rhs=rhs,
                        start=(dc == 0), stop=(dc == NDC - 1))
                dst = (qT if tens == "q" else kT)[mc][
                    :, sh * 1024 + sc * 512: sh * 1024 + (sc + 1) * 512]
                if tens == "q":
                    nc.vector.tensor_scalar_add(
                        out=dst, in0=ps, scalar1=bq_sb[:, mc:mc + 1])
                else:
                    nc.vector.tensor_copy(out=dst, in_=ps)

            def proj_v(kb):
                """Project v key-chunk kb: out [s 128, m 256]; x stationary."""
                sh, sc = divmod(kb, 8)
                ps = psp.tile([128, 512], f32, tag="ps", name="ps")
                for dc in range(NDC):
                    nc.tensor.matmul(
                        ps[:, 0:M],
                        lhsT=xtiles[("v", dc, sh)][:, sc * 128:(sc + 1) * 128],
                        rhs=wv_sb[:, dc, :],
                        start=(dc == 0), stop=(dc == NDC - 1))
                for h in range(HPC):
                    nc.vector.tensor_copy(
                        out=v_sb[:, kb, h, 0:DK],
                        in_=ps[:, h * DK:(h + 1) * DK])

            e_tiles = {}
            SPLIT_U = 7  # last unit: tiles cover [2 kb x 512 queries] so the
                         # first query-half closes out while the second exps

            def qk_tile(u, t):
                """Scores^T tile [keys 128, 1024] + exp -> e (bf16)."""
                ih, h = divmod(u, HPC)
                mc, off = divmod(h, 2)
                off *= DK
                qk = qkp.tile([128, EXPW], f32, tag="qk", name="qk")
                if u == SPLIT_U:
                    qh, kbp = divmod(t, 8)
                    blocks = [(2 * kbp + j, qh * 512, j * 512) for j in range(2)]
                else:
                    blocks = [(t, ha * 512, ha * 512) for ha in range(2)]
                for kb, qoff, coff in blocks:
                    nc.tensor.matmul(
                        qk[:, coff:coff + 512],
                        lhsT=kT[mc][off:off + DK, kb * 128:(kb + 1) * 128],
                        rhs=qT[mc][off:off + DK,
                                   ih * IW + qoff: ih * IW + qoff + 512],
                        start=True, stop=True)
                e = ep.tile([128, EXPW], bf16, tag="e", name=f"e{u}_{t}")
                nc.scalar.activation(out=e, in_=qk, func=AF.Exp,
                                     scale=1.0 / np.sqrt(DK))
                e_tiles[(u, t)] = e

            def e_slice(u, kb, qc):
                """lhsT slice [keys 128, queries 128] of unit u's e tiles."""
                if u == SPLIT_U:
                    t = (qc // 4) * 8 + kb // 2
                    off = (kb % 2) * 512 + (qc % 4) * 128
                else:
                    t, off = kb, qc * 128
                return e_tiles[(u, t)][:, off:off + 128]

            pv_psum = {}

            def pv_qc(u, qc):
                """ctx[q 128, dk+1] for queries qc of unit u; accumulate all kb.
                Then normalize into stage (per-partition scalar multiply)."""
                ih, h = divmod(u, HPC)
                qg, q4 = divmod(qc, 4)
                if q4 == 0:
                    pv_psum[(u, qg)] = pvp.tile([128, 4, DK + 1], f32,
                                                tag="pv", name="pv")
                pv = pv_psum[(u, qg)]
                for kb in range(NKB):
                    nc.tensor.matmul(
                        pv[:, q4, :],
                        lhsT=e_slice(u, kb, qc),
                        rhs=v_sb[:, kb, h, :],
                        start=(kb == 0), stop=(kb == NKB - 1))
                inv = invp.tile([128, 1], f32, tag="inv", name="inv")
                nc.vector.reciprocal(out=inv, in_=pv[:, q4, DK:DK + 1])
                nc.vector.tensor_scalar_mul(
                    out=stage[ih][qc][:, h * DK:(h + 1) * DK],
                    in0=pv[:, q4, 0:DK], scalar1=inv)

            def t_qc(ih, qc, mc):
                """Transpose normalized ctx block [q 128, m 128] -> ctx_t."""
                ps = psp.tile([128, 512], bf16, tag="ps", name="tp")
                nc.tensor.transpose(
                    ps[:, 0:128],
                    in_=stage[ih][qc][:, mc * 128:(mc + 1) * 128],
                    identity=ident)
                nc.vector.tensor_copy(
                    out=ctx_t[ih][mc][:, qc * 128:(qc + 1) * 128],
                    in_=ps[:, 0:128])

            def wo_qc(ih, qc):
                """Output projection for query chunk qc of half ih."""
                for nh in range(2):
                    ps = psp.tile([128, 512], f32, tag="ps", name="wops")
                    for mc in range(NMC):
                        nc.tensor.matmul(
                            ps,
                            lhsT=ctx_t[ih][mc][:, qc * 128:(qc + 1) * 128],
                            rhs=wo_sb[:, mc, nh * 512:(nh + 1) * 512],
                            start=(mc == 0), stop=(mc == NMC - 1))
                    os_ = ostp.tile([128, 512], f32, tag="ost", name="ost")
                    nc.vector.tensor_copy(out=os_, in_=ps)
                    nc.sync.dma_start(
                        out=out_d[(ih * 8 + qc) * 128:(ih * 8 + qc + 1) * 128,
                                  nh * 512:(nh + 1) * 512],
                        in_=os_)

            # ---- static filler schedule ----
            PK = lambda tens, sh, mc, sc: (lambda: proj_mk(tens, sh, mc, sc))
            PJV = lambda kb: (lambda: proj_v(kb))
            PV = lambda u, qc: (lambda: pv_qc(u, qc))
            T = lambda ih, mc: (lambda: [t_qc(ih, qc, mc) for qc in range(8)])
            T1q = lambda qc: (lambda: t_qc(1, qc, 1))
            WO = lambda ih, qc: (lambda: wo_qc(ih, qc))

            FILL = {
                (0, 2): [PK("k", 0, 0, 1)],
                (0, 4): [PK("k", 0, 1, 0)], (0, 6): [PK("k", 0, 1, 1)],
                (0, 7): [PK("k", 1, 0, 0)], (0, 9): [PK("k", 1, 0, 1)],
                (0, 11): [PK("k", 1, 1, 0)], (0, 13): [PK("k", 1, 1, 1)],
                (0, 14): [PK("q", 0, 1, 0)], (0, 15): [PK("q", 0, 1, 1)],
                (1, 0): [PJV(0)], (1, 2): [PJV(1)], (1, 4): [PJV(2)],
                (1, 5): [PJV(8)], (1, 6): [PJV(3)], (1, 8): [PJV(4)],
                (1, 9): [PJV(9)], (1, 10): [PJV(5)], (1, 12): [PJV(6)],
                (1, 13): [PJV(10)], (1, 14): [PJV(7)], (1, 15): [PJV(11)],
                (2, 0): [PJV(12)], (2, 2): [PJV(13)],
                (2, 4): [PJV(14)], (2, 6): [PJV(15)],
                (2, 8): [PV(0, 0)], (2, 9): [PV(0, 1)], (2, 10): [PV(0, 2)],
                (2, 11): [PV(0, 3)], (2, 12): [PV(0, 4)], (2, 13): [PV(0, 5)],
                (2, 14): [PV(0, 6)], (2, 15): [PV(0, 7)],
                (3, 0): [PK("q", 1, 0, 0)], (3, 2): [PK("q", 1, 0, 1)],
                (3, 4): [PK("q", 1, 1, 0)], (3, 6): [PK("q", 1, 1, 1)],
                (3, 8): [PV(1, 0)], (3, 9): [PV(1, 1)], (3, 10): [PV(1, 2)],
                (3, 11): [PV(1, 3)], (3, 12): [PV(1, 4)], (3, 13): [PV(1, 5)],
                (3, 14): [PV(1, 6)], (3, 15): [PV(1, 7)],
                (4, 0): [T(0, 0)],
                (4, 1): [PV(2, 0)], (4, 2): [PV(2, 1)], (4, 3): [PV(2, 2)],
                (4, 4): [PV(2, 3)], (4, 5): [PV(2, 4)], (4, 6): [PV(2, 5)],
                (4, 7): [PV(2, 6)], (4, 8): [PV(2, 7)],
                (4, 9): [PV(3, 0)], (4, 10): [PV(3, 1)], (4, 11): [PV(3, 2)],
                (4, 12): [PV(3, 3)], (4, 13): [PV(3, 4)], (4, 14): [PV(3, 5)],
                (4, 15): [PV(3, 6)],
                (5, 0): [PV(3, 7)], (5, 1): [T(0, 1)],
                (5, 2): [WO(0, 0)], (5, 4): [WO(0, 1)], (5, 6): [WO(0, 2)],
                (5, 8): [WO(0, 3)], (5, 10): [WO(0, 4)], (5, 12): [WO(0, 5)],
                (5, 14): [WO(0, 6)], (5, 15): [WO(0, 7)],
                (6, 0): [PV(4, 0)], (6, 1): [PV(4, 1)], (6, 2): [PV(4, 2)],
                (6, 3): [PV(4, 3)], (6, 4): [PV(4, 4)], (6, 5): [PV(4, 5)],
                (6, 6): [PV(4, 6)], (6, 7): [PV(4, 7)],
                (6, 8): [PV(5, 0)], (6, 9): [PV(5, 1)], (6, 10): [PV(5, 2)],
                (6, 11): [PV(5, 3)], (6, 12): [PV(5, 4)], (6, 13): [PV(5, 5)],
                (6, 14): [PV(5, 6)], (6, 15): [PV(5, 7)],
                (7, 0): [T(1, 0)],
                (7, 1): [PV(6, 0)], (7, 2): [PV(6, 1)], (7, 3): [PV(6, 2)],
                (7, 4): [PV(6, 3)], (7, 5): [PV(6, 4)], (7, 6): [PV(6, 5)],
                (7, 7): [PV(6, 6)], (7, 8): [PV(6, 7)],
                (7, 9): [PV(7, 0)],
                (7, 10): [PV(7, 1), T1q(0)],
                (7, 11): [PV(7, 2), T1q(1), WO(1, 0)],
                (7, 12): [PV(7, 3), T1q(2), WO(1, 1)],
                (7, 13): [T1q(3), WO(1, 2)],
                (7, 14): [WO(1, 3)],
            }

            # ---- main pipeline ----
            # prologue: first projections (DMA-gated)
            proj_mk("k", 0, 0, 0)
            proj_mk("q", 0, 0, 0)
            proj_mk("q", 0, 0, 1)
            # 8 attention units paced by the exp stream
            for u in range(8):
                for t in range(NT):
                    qk_tile(u, t)
                    for fn in FILL.get((u, t), []):
                        fn()
            # tail: close out the last unit's second query-half
            pv_qc(7, 4)
            pv_qc(7, 5)
            t_qc(1, 4, 1)
            wo_qc(1, 4)
            pv_qc(7, 6)
            t_qc(1, 5, 1)
            wo_qc(1, 5)
            pv_qc(7, 7)
            t_qc(1, 6, 1)
            wo_qc(1, 6)
            t_qc(1, 7, 1)
            wo_qc(1, 7)

    nc.compile()
    return nc


def _get_nc(debug=False):
    key = ("nc", debug)
    if key not in _cached:
        _cached[key] = _build(debug)
    return _cached[key]


def _get_runner():
    """Build (once) a jitted 8-core SPMD executable mirroring
    bass2jax.run_bass_via_pjrt, reusable across calls for benchmarking."""
    if "runner" in _cached:
        return _cached["runner"]
    import jax
    import jax.numpy as jnp
    from jax.experimental.shard_map import shard_map
    from jax.sharding import Mesh, PartitionSpec
    import concourse.mybir as mybir
    from concourse import bass2jax

    bass2jax.install_neuronx_cc_hook()
    nc = _get_nc()
    assert nc.dbg_addr is None
    partition_name = nc.partition_id_tensor.name if nc.partition_id_tensor else None

    in_names, out_names, out_avals, zero_outs = [], [], [], []
    for alloc in nc.m.functions[0].allocations:
        if not isinstance(alloc, mybir.MemoryLocationSet):
            continue
        name = alloc.memorylocations[0].name
        if alloc.kind == "ExternalInput":
            if name != partition_name:
                in_names.append(name)
        elif alloc.kind == "ExternalOutput":
            out_names.append(name)
            shape = tuple(alloc.tensor_shape)
            dtype = mybir.dt.np(alloc.dtype)
            out_avals.append(jax.core.ShapedArray(shape, dtype))
            zero_outs.append(np.zeros(shape, dtype))
    n_params = len(in_names)
    all_in_names = in_names + out_names
    if partition_name is not None:
        all_in_names = all_in_names + [partition_name]
    donate = tuple(range(n_params, n_params + len(out_names)))

    def _body(*args):
        operands = list(args)
        if partition_name is not None:
            operands.append(bass2jax.partition_id_tensor())
        outs = bass2jax._bass_exec_p.bind(
            *operands,
            out_avals=tuple(out_avals),
            in_names=tuple(all_in_names),
            out_names=tuple(out_names),
            lowering_input_output_aliases=(),
            sim_require_finite=True,
            sim_require_nnan=True,
            nc=nc,
        )
        return tuple(outs)

    devices = jax.devices()[:NC]
    mesh = Mesh(np.asarray(devices), ("core",))
    nin = n_params + len(out_names)
    sharded = jax.jit(
        shard_map(
            _body,
            mesh=mesh,
            in_specs=(PartitionSpec("core"),) * nin,
            out_specs=(PartitionSpec("core"),) * len(out_names),
            check_rep=False,
        ),
        donate_argnums=donate,
        keep_unused=True,
    )

    def run(in_maps):
        concat_in = [
            np.concatenate([np.asarray(in_maps[c][n]) for c in range(NC)], axis=0)
            for n in in_names
        ]
        concat_zeros = [
            np.zeros((NC * z.shape[0], *z.shape[1:]), z.dtype) for z in zero_outs
        ]
        out_arrs = sharded(*concat_in, *concat_zeros)
        return [
            {
                n: np.asarray(out_arrs[i]).reshape(NC, *out_avals[i].shape)[c]
                for i, n in enumerate(out_names)
            }
            for c in range(NC)
        ]

    _cached["runner"] = (run, sharded, in_names, out_names, out_avals, zero_outs)
    return _cached["runner"]


def _make_in_maps(query, key, value, Wq, bq, Wk, bk, Wv, bv, Wo, bo):
    import ml_dtypes
    bf16 = ml_dtypes.bfloat16

    query = np.asarray(query, dtype=np.float32)
    key = np.asarray(key, dtype=np.float32)
    value = np.asarray(value, dtype=np.float32)
    Wq, Wk, Wv, Wo = (np.asarray(a, dtype=np.float32) for a in (Wq, Wk, Wv, Wo))
    bq = np.asarray(bq, dtype=np.float32)
    B = query.shape[0]
    ident = np.eye(128, dtype=bf16)

    xqT = [np.ascontiguousarray(query[b].T).astype(bf16) for b in range(B)]
    xkT = [np.ascontiguousarray(key[b].T).astype(bf16) for b in range(B)]
    xvT = [np.ascontiguousarray(value[b].T).astype(bf16) for b in range(B)]

    in_maps = []
    for c in range(NC):
        b, hg = divmod(c, NC // B)
        sl = slice(hg * M, (hg + 1) * M)
        in_maps.append(
            {
                "xqT": xqT[b],
                "xkT": xkT[b],
                "xvT": xvT[b],
                "wq": np.ascontiguousarray(Wq[:, sl]).astype(bf16),
                "wk": np.ascontiguousarray(Wk[:, sl]).astype(bf16),
                "wv": np.ascontiguousarray(Wv[:, sl]).astype(bf16),
                "wo": np.ascontiguousarray(Wo[sl, :]).astype(bf16),
                "bq": np.ascontiguousarray(bq[sl]),
                "ident": ident,
            }
        )
    return in_maps


def kernel(query, key, value, Wq, bq, Wk, bk, Wv, bv, Wo, bo):
    in_maps = _make_in_maps(query, key, value, Wq, bq, Wk, bk, Wv, bv, Wo, bo)
    run = _get_runner()[0]
    results = run(in_maps)

    B = np.asarray(query).shape[0]
    bo = np.asarray(bo, dtype=np.float32)
    bv = np.asarray(bv, dtype=np.float32)
    Wo_f = np.asarray(Wo, dtype=np.float32)
    base = bo + bv @ Wo_f  # bv contributes exactly bv @ Wo (sum of attn = 1)
    full = np.zeros((B, S, D), np.float32)
    for b in range(B):
        acc = np.zeros((S, D), np.float32)
        for g in range(NC // B):
            acc += results[b * (NC // B) + g]["out"]
        full[b] = acc + base[None, :]
    return full


# revision 17
# speedup vs baseline: 1.4263x; 1.0194x over previous
"""Multi-head attention (B=2, S=2048, D=1024, H=16) on 8 TRN2 NeuronCores.

Sharding: (batch, head-group) — core c handles batch c//4 and heads
[4*(c%4), 4*(c%4)+4). Each core projects its batch's tokens onto its 4 heads'
column-shards of Wq/Wk/Wv, runs attention for those heads, and multiplies by
its row-shard of Wo, producing a partial [S, D] output. The host sums the 4
partials per batch and adds (bo + bv @ Wo). bk is dropped entirely (a key
bias shifts every score of a query by the same constant, which softmax
cancels); bv contributes bv @ Wo to the output because attention weights sum
to one.

Device design notes:
  - All matmuls run in bf16 (1 cycle/row at any free size on the PE cost
    model); inputs and weights are cast to bf16 on the host, halving input
    DMA. PSUM accumulation stays f32.
  - Q/K are projected feature-major (out [m, s]; W stationary). V is
    projected token-major (x stationary, Wv moving) giving v in [keys, dk]
    layout directly — no V transpose pass. A constant-1 column appended to
    each per-(key-chunk, head) V block produces the softmax denominator
    inside the PV matmul.
  - Scores are computed transposed (S^T [key, query]) in [128, 2048] PSUM
    tiles (4 banks) so each Exp activation covers 2048 elements/partition,
    minimizing Act-engine fixed overhead. Act is the #2 engine (~121us).
  - PV is computed with queries on the OUTPUT partitions: out[q, dk+1] +=
    e_chunk^T-slice @ v_chunk. Free dim is 65 instead of 512, so PV costs
    half the baseline's PE cycles. Softmax normalization becomes a
    per-partition scalar multiply (DVE reciprocal of the denominator column
    + tensor_scalar_mul) — no partition broadcast needed.
  - ctx [q, m] is then PE-transposed per 128x128 block into ctx^T [m, q]
    for the Wo projection (contraction over m).
  - Emission order IS each engine's execution order. The schedule runs 8
    attention units (ih half x head) paced by the Act engine's exp stream;
    projections, PV of earlier units, transposes and Wo chunks are
    interleaved between score tiles via a static filler table.
  - PSUM: 2x1-bank pool (projections/transposes/Wo), 1x4-bank score tile,
    2x1-bank PV accumulators ([128, 4, 65] f32). Total exactly 8 banks.
"""

import numpy as np

S = 2048          # sequence length
D = 1024          # model dim
HPC = 4           # heads per core
DK = 64           # head dim
M = HPC * DK      # per-core projection width = 256
NC = 8            # cores
IW = 1024         # attention query width per ih-half
NDC = D // 128    # 8 contraction chunks
NMC = M // 128    # 2 m-chunks (head pairs)
NKB = S // 128    # 16 key chunks
EXPW = 1024       # exp tile width
NT = 16 * IW // EXPW  # qk/exp tiles per unit = 16

_cached = {}


def _build(debug=False):
    import concourse.bass as bass
    import concourse.bacc as bacc
    import concourse.tile as tile
    import concourse.mybir as mybir
    from contextlib import ExitStack

    f32 = mybir.dt.float32
    bf16 = mybir.dt.bfloat16
    AF = mybir.ActivationFunctionType

    nc = bacc.Bacc(
        "TRN2",
        target_bir_lowering=False,
        debug=False,
        enable_asserts=False,
        num_devices=NC,
    )

    # DRAM I/O (per-core shapes)
    xqT_d = nc.dram_tensor("xqT", [D, S], bf16, kind="ExternalInput").ap()
    xkT_d = nc.dram_tensor("xkT", [D, S], bf16, kind="ExternalInput").ap()
    xvT_d = nc.dram_tensor("xvT", [D, S], bf16, kind="ExternalInput").ap()
    wq_d = nc.dram_tensor("wq", [D, M], bf16, kind="ExternalInput").ap()
    wk_d = nc.dram_tensor("wk", [D, M], bf16, kind="ExternalInput").ap()
    wv_d = nc.dram_tensor("wv", [D, M], bf16, kind="ExternalInput").ap()
    wo_d = nc.dram_tensor("wo", [M, D], bf16, kind="ExternalInput").ap()
    bq_d = nc.dram_tensor("bq", [M], f32, kind="ExternalInput").ap()
    ident_d = nc.dram_tensor("ident", [128, 128], bf16, kind="ExternalInput").ap()
    out_d = nc.dram_tensor("out", [S, D], f32, kind="ExternalOutput").ap()

    with tile.TileContext(nc) as tc:
        with ExitStack() as st:
            wp = st.enter_context(tc.tile_pool(name="wp", bufs=1))
            xt = st.enter_context(tc.tile_pool(name="xt", bufs=14))
            qkt = st.enter_context(tc.tile_pool(name="qkt", bufs=1))
            vp = st.enter_context(tc.tile_pool(name="vp", bufs=1))
            ep = st.enter_context(tc.tile_pool(name="ep", bufs=40))
            stp = st.enter_context(tc.tile_pool(name="stp", bufs=1))
            ctp = st.enter_context(tc.tile_pool(name="ctp", bufs=1))
            invp = st.enter_context(tc.tile_pool(name="invp", bufs=4))
            ostp = st.enter_context(tc.tile_pool(name="ostp", bufs=4))
            # PSUM: exactly 8 banks
            psp = st.enter_context(tc.tile_pool(name="psp", bufs=2, space="PSUM"))
            qkp = st.enter_context(tc.tile_pool(name="qkp", bufs=2, space="PSUM"))
            pvp = st.enter_context(tc.tile_pool(name="pvp", bufs=2, space="PSUM"))

            wq_sb = wp.tile([128, NDC, M], bf16, tag="wq")
            wk_sb = wp.tile([128, NDC, M], bf16, tag="wk")
            wv_sb = wp.tile([128, NDC, M], bf16, tag="wv")
            wo_sb = wp.tile([128, NMC, D], bf16, tag="wo")
            bq_sb = wp.tile([128, NMC], f32, tag="bq")
            ident = wp.tile([128, 128], bf16, tag="ident")
            qT = [qkt.tile([128, S], bf16, tag=f"qT{m}", name=f"qT{m}")
                  for m in range(NMC)]
            kT = [qkt.tile([128, S], bf16, tag=f"kT{m}", name=f"kT{m}")
                  for m in range(NMC)]
            # v in [keys, head, dk+1] layout; col DK is the constant 1
            v_sb = vp.tile([128, NKB, HPC, DK + 1], bf16, tag="v")
            stage = [[stp.tile([128, M], bf16, tag=f"st{i}{q}", name=f"st{i}{q}")
                      for q in range(8)] for i in range(2)]
            ctx_t = [[ctp.tile([128, IW], bf16, tag=f"ct{i}{m}", name=f"ct{i}{m}")
                      for m in range(NMC)] for i in range(2)]

            nc.vector.memset(v_sb[:, :, :, DK:DK + 1], 1.0)

            w_r = lambda ap: ap.rearrange("(n p) m -> p n m", p=128)

            # ---- input DMA emission (order = SP.SEQ issue order) ----
            # x tensor-halves load as 4 chunked DMAs of [128, 2 dc, 1024]
            # each — few DMAs (HWDGE is a serial ~625ns/DMA resource) but
            # still streamable by the projections.
            xtiles = {}   # (tensor, dc-pair, sh) -> [128, 2, 1024] tile

            def load_x(tens, dram, sh):
                for dp in range(NDC // 2):
                    t = xt.tile([128, 2, 1024], bf16, tag="x",
                                name=f"x{tens}{dp}{sh}")
                    nc.sync.dma_start(
                        out=t,
                        in_=dram[dp * 256:(dp + 1) * 256,
                                 sh * 1024:(sh + 1) * 1024].rearrange(
                                     "(n p) s -> p n s", p=128))
                    xtiles[(tens, dp, sh)] = t

            def xap(tens, dc, sh):
                return xtiles[(tens, dc // 2, sh)][:, dc % 2, :]

            nc.sync.dma_start(out=wk_sb, in_=w_r(wk_d))
            nc.sync.dma_start(out=wq_sb, in_=w_r(wq_d))
            nc.sync.dma_start(out=bq_sb, in_=bq_d.rearrange("(n p) -> p n", p=128))
            nc.sync.dma_start(out=ident, in_=ident_d)
            load_x("k", xkT_d, 0)
            load_x("q", xqT_d, 0)
            load_x("k", xkT_d, 1)
            nc.sync.dma_start(out=wv_sb, in_=w_r(wv_d))
            load_x("v", xvT_d, 0)
            load_x("v", xvT_d, 1)
            load_x("q", xqT_d, 1)
            nc.sync.dma_start(out=wo_sb, in_=wo_d.rearrange("(g p) n -> p g n", p=128))

            # ---- emission helpers ----
            def proj_mk(tens, sh, mc, sc):
                """Project q/k chunk: out [m 128, s 512]; W stationary."""
                w_sb = wq_sb if tens == "q" else wk_sb
                ps = psp.tile([128, 512], f32, tag="ps", name="ps")
                for dc in range(NDC):
                    nc.tensor.matmul(
                        ps,
                        lhsT=w_sb[:, dc, mc * 128:(mc + 1) * 128],
                        rhs=xap(tens, dc, sh)[:, sc * 512:(sc + 1) * 512],
                        start=(dc == 0), stop=(dc == NDC - 1))
                dst = (qT if tens == "q" else kT)[mc][
                    :, sh * 1024 + sc * 512: sh * 1024 + (sc + 1) * 512]
                if tens == "q":
                    nc.vector.tensor_scalar_add(
                        out=dst, in0=ps, scalar1=bq_sb[:, mc:mc + 1])
                else:
                    nc.vector.tensor_copy(out=dst, in_=ps)

            def proj_v(kb):
                """Project v key-chunk kb: out [s 128, m 256]; x stationary."""
                sh, sc = divmod(kb, 8)
                ps = psp.tile([128, 512], f32, tag="ps", name="ps")
                for dc in range(NDC):
                    nc.tensor.matmul(
                        ps[:, 0:M],
                        lhsT=xap("v", dc, sh)[:, sc * 128:(sc + 1) * 128],
                        rhs=wv_sb[:, dc, :],
                        start=(dc == 0), stop=(dc == NDC - 1))
                for h in range(HPC):
                    nc.vector.tensor_copy(
                        out=v_sb[:, kb, h, 0:DK],
                        in_=ps[:, h * DK:(h + 1) * DK])

            e_tiles = {}
            SPLIT_U = 7  # last unit: tiles cover [4 kb x 256 queries] so each
                         # query-quarter closes out while later quarters exp

            def qk_tile(u, t):
                """Scores^T tile [keys 128, 1024] + exp -> e (bf16)."""
                ih, h = divmod(u, HPC)
                mc, off = divmod(h, 2)
                off *= DK
                qk = qkp.tile([128, EXPW], f32, tag="qk", name="qk")
                if u == SPLIT_U:
                    qq, kbp = divmod(t, 4)
                    blocks = [(4 * kbp + j, qq * 256, j * 256, 256)
                              for j in range(4)]
                else:
                    blocks = [(t, ha * 512, ha * 512, 512) for ha in range(2)]
                for kb, qoff, coff, w in blocks:
                    nc.tensor.matmul(
                        qk[:, coff:coff + w],
                        lhsT=kT[mc][off:off + DK, kb * 128:(kb + 1) * 128],
                        rhs=qT[mc][off:off + DK,
                                   ih * IW + qoff: ih * IW + qoff + w],
                        start=True, stop=True)
                e = ep.tile([128, EXPW], bf16, tag="e", name=f"e{u}_{t}")
                nc.scalar.activation(out=e, in_=qk, func=AF.Exp,
                                     scale=1.0 / np.sqrt(DK))
                e_tiles[(u, t)] = e

            def e_slice(u, kb, qc):
                """lhsT slice [keys 128, queries 128] of unit u's e tiles."""
                if u == SPLIT_U:
                    t = (qc // 2) * 4 + kb // 4
                    off = (kb % 4) * 256 + (qc % 2) * 128
                else:
                    t, off = kb, qc * 128
                return e_tiles[(u, t)][:, off:off + 128]

            pv_psum = {}

            def pv_qc(u, qc):
                """ctx[q 128, dk+1] for queries qc of unit u; accumulate all kb.
                Then normalize into stage (per-partition scalar multiply)."""
                ih, h = divmod(u, HPC)
                qg, q4 = divmod(qc, 4)
                if q4 == 0:
                    pv_psum[(u, qg)] = pvp.tile([128, 4, DK + 1], f32,
                                                tag="pv", name="pv")
                pv = pv_psum[(u, qg)]
                for kb in range(NKB):
                    nc.tensor.matmul(
                        pv[:, q4, :],
                        lhsT=e_slice(u, kb, qc),
                        rhs=v_sb[:, kb, h, :],
                        start=(kb == 0), stop=(kb == NKB - 1))
                inv = invp.tile([128, 1], f32, tag="inv", name="inv")
                nc.vector.reciprocal(out=inv, in_=pv[:, q4, DK:DK + 1])
                nc.vector.tensor_scalar_mul(
                    out=stage[ih][qc][:, h * DK:(h + 1) * DK],
                    in0=pv[:, q4, 0:DK], scalar1=inv)

            def t_qc(ih, qc, mc):
                """Transpose normalized ctx block [q 128, m 128] -> ctx_t."""
                ps = psp.tile([128, 512], bf16, tag="ps", name="tp")
                nc.tensor.transpose(
                    ps[:, 0:128],
                    in_=stage[ih][qc][:, mc * 128:(mc + 1) * 128],
                    identity=ident)
                nc.vector.tensor_copy(
                    out=ctx_t[ih][mc][:, qc * 128:(qc + 1) * 128],
                    in_=ps[:, 0:128])

            def wo_qc(ih, qc):
                """Output projection for query chunk qc of half ih."""
                for nh in range(2):
                    ps = psp.tile([128, 512], f32, tag="ps", name="wops")
                    for mc in range(NMC):
                        nc.tensor.matmul(
                            ps,
                            lhsT=ctx_t[ih][mc][:, qc * 128:(qc + 1) * 128],
                            rhs=wo_sb[:, mc, nh * 512:(nh + 1) * 512],
                            start=(mc == 0), stop=(mc == NMC - 1))
                    os_ = ostp.tile([128, 512], f32, tag="ost", name="ost")
                    nc.vector.tensor_copy(out=os_, in_=ps)
                    nc.sync.dma_start(
                        out=out_d[(ih * 8 + qc) * 128:(ih * 8 + qc + 1) * 128,
                                  nh * 512:(nh + 1) * 512],
                        in_=os_)

            # ---- static filler schedule ----
            PK = lambda tens, sh, mc, sc: (lambda: proj_mk(tens, sh, mc, sc))
            PJV = lambda kb: (lambda: proj_v(kb))
            PV = lambda u, qc: (lambda: pv_qc(u, qc))
            T = lambda ih, mc: (lambda: [t_qc(ih, qc, mc) for qc in range(8)])
            T1q = lambda qc: (lambda: t_qc(1, qc, 1))
            WO = lambda ih, qc: (lambda: wo_qc(ih, qc))

            FILL = {
                (0, 1): [PK("k", 0, 1, 0)], (0, 3): [PK("k", 0, 1, 1)],
                (0, 5): [PK("q", 0, 1, 0)],
                (0, 7): [PK("k", 1, 0, 0)], (0, 9): [PK("k", 1, 0, 1)],
                (0, 11): [PK("q", 0, 1, 1)],
                (0, 13): [PK("k", 1, 1, 0)],
                (1, 1): [PK("k", 1, 1, 1)],
                (1, 0): [PJV(0)], (1, 2): [PJV(1)], (1, 4): [PJV(2)],
                (1, 6): [PJV(3)], (1, 8): [PJV(4)], (1, 10): [PJV(5)],
                (1, 12): [PJV(6)], (1, 14): [PJV(7)],
                (2, 0): [PJV(8)], (2, 1): [PJV(9)], (2, 2): [PJV(10)],
                (2, 3): [PJV(11)], (2, 4): [PJV(12)], (2, 5): [PJV(13)],
                (2, 6): [PJV(14)], (2, 7): [PJV(15)],
                (2, 8): [PV(0, 0)], (2, 9): [PV(0, 1)], (2, 10): [PV(0, 2)],
                (2, 11): [PV(0, 3)], (2, 12): [PV(0, 4)], (2, 13): [PV(0, 5)],
                (2, 14): [PV(0, 6)], (2, 15): [PV(0, 7)],
                (3, 0): [PK("q", 1, 0, 0)], (3, 2): [PK("q", 1, 0, 1)],
                (3, 4): [PK("q", 1, 1, 0)], (3, 6): [PK("q", 1, 1, 1)],
                (3, 8): [PV(1, 0)], (3, 9): [PV(1, 1)], (3, 10): [PV(1, 2)],
                (3, 11): [PV(1, 3)], (3, 12): [PV(1, 4)], (3, 13): [PV(1, 5)],
                (3, 14): [PV(1, 6)], (3, 15): [PV(1, 7)],
                (4, 0): [PV(2, 0)], (4, 1): [PV(2, 1)], (4, 2): [PV(2, 2)],
                (4, 3): [PV(2, 3)], (4, 4): [PV(2, 4)], (4, 5): [PV(2, 5)],
                (4, 6): [PV(2, 6)], (4, 7): [PV(2, 7)],
                (4, 8): [PV(3, 0)], (4, 9): [PV(3, 1)], (4, 10): [PV(3, 2)],
                (4, 11): [PV(3, 3)], (4, 12): [PV(3, 4)], (4, 13): [PV(3, 5)],
                (4, 14): [PV(3, 6)], (4, 15): [PV(3, 7)],
                (5, 0): [T(0, 0)], (5, 1): [T(0, 1)],
                (5, 2): [WO(0, 0)], (5, 4): [WO(0, 1)], (5, 6): [WO(0, 2)],
                (5, 8): [WO(0, 3)], (5, 10): [WO(0, 4)], (5, 12): [WO(0, 5)],
                (5, 14): [WO(0, 6)], (5, 15): [WO(0, 7)],
                (6, 0): [PV(4, 0)], (6, 1): [PV(4, 1)], (6, 2): [PV(4, 2)],
                (6, 3): [PV(4, 3)], (6, 4): [PV(4, 4)], (6, 5): [PV(4, 5)],
                (6, 6): [PV(4, 6)], (6, 7): [PV(4, 7)],
                (6, 8): [PV(5, 0)], (6, 9): [PV(5, 1)], (6, 10): [PV(5, 2)],
                (6, 11): [PV(5, 3)], (6, 12): [PV(5, 4)], (6, 13): [PV(5, 5)],
                (6, 14): [PV(5, 6)], (6, 15): [PV(5, 7)],
                (7, 0): [T(1, 0)],
                (7, 1): [PV(6, 0)], (7, 2): [PV(6, 1)], (7, 3): [PV(6, 2)],
                (7, 4): [PV(6, 3)],
                (7, 5): [PV(7, 0)],
                (7, 6): [PV(7, 1), T1q(0)],
                (7, 7): [PV(6, 4), WO(1, 0)],
                (7, 8): [PV(6, 5), T1q(1)],
                (7, 9): [PV(6, 6), WO(1, 1)],
                (7, 10): [PV(6, 7), PV(7, 2)],
                (7, 11): [PV(7, 3), T1q(2)],
                (7, 12): [WO(1, 2), T1q(3)],
                (7, 13): [WO(1, 3), PV(7, 4)],
                (7, 14): [PV(7, 5), T1q(4)],
                (7, 15): [WO(1, 4)],
            }

            # ---- main pipeline ----
            # prologue: first projections (DMA-gated)
            proj_mk("k", 0, 0, 0)
            proj_mk("k", 0, 0, 1)
            proj_mk("q", 0, 0, 0)
            proj_mk("q", 0, 0, 1)
            # 8 attention units paced by the exp stream
            for u in range(8):
                for t in range(NT):
                    qk_tile(u, t)
                    for fn in FILL.get((u, t), []):
                        fn()
            # tail: close out the last unit's final query-quarters
            pv_qc(7, 6)
            t_qc(1, 5, 1)
            wo_qc(1, 5)
            pv_qc(7, 7)
            t_qc(1, 6, 1)
            wo_qc(1, 6)
            t_qc(1, 7, 1)
            wo_qc(1, 7)

    nc.compile()
    return nc


def _get_nc(debug=False):
    key = ("nc", debug)
    if key not in _cached:
        _cached[key] = _build(debug)
    return _cached[key]


def _get_runner():
    """Build (once) a jitted 8-core SPMD executable mirroring
    bass2jax.run_bass_via_pjrt, reusable across calls for benchmarking."""
    if "runner" in _cached:
        return _cached["runner"]
    import jax
    import jax.numpy as jnp
    from jax.experimental.shard_map import shard_map
    from jax.sharding import Mesh, PartitionSpec
    import concourse.mybir as mybir
    from concourse import bass2jax

    bass2jax.install_neuronx_cc_hook()
    nc = _get_nc()
    assert nc.dbg_addr is None
    partition_name = nc.partition_id_tensor.name if nc.partition_id_tensor else None

    in_names, out_names, out_avals, zero_outs = [], [], [], []
    for alloc in nc.m.functions[0].allocations:
        if not isinstance(alloc, mybir.MemoryLocationSet):
            continue
        name = alloc.memorylocations[0].name
        if alloc.kind == "ExternalInput":
            if name != partition_name:
                in_names.append(name)
        elif alloc.kind == "ExternalOutput":
            out_names.append(name)
            shape = tuple(alloc.tensor_shape)
            dtype = mybir.dt.np(alloc.dtype)
            out_avals.append(jax.core.ShapedArray(shape, dtype))
            zero_outs.append(np.zeros(shape, dtype))
    n_params = len(in_names)
    all_in_names = in_names + out_names
    if partition_name is not None:
        all_in_names = all_in_names + [partition_name]
    donate = tuple(range(n_params, n_params + len(out_names)))

    def _body(*args):
        operands = list(args)
        if partition_name is not None:
            operands.append(bass2jax.partition_id_tensor())
        outs = bass2jax._bass_exec_p.bind(
            *operands,
            out_avals=tuple(out_avals),
            in_names=tuple(all_in_names),
            out_names=tuple(out_names),
            lowering_input_output_aliases=(),
            sim_require_finite=True,
            sim_require_nnan=True,
            nc=nc,
        )
        return tuple(outs)

    devices = jax.devices()[:NC]
    mesh = Mesh(np.asarray(devices), ("core",))
    nin = n_params + len(out_names)
    sharded = jax.jit(
        shard_map(
            _body,
            mesh=mesh,
            in_specs=(PartitionSpec("core"),) * nin,
            out_specs=(PartitionSpec("core"),) * len(out_names),
            check_rep=False,
        ),
        donate_argnums=donate,
        keep_unused=True,
    )

    def run(in_maps):
        concat_in = [
            np.concatenate([np.asarray(in_maps[c][n]) for c in range(NC)], axis=0)
            for n in in_names
        ]
        concat_zeros = [
            np.zeros((NC * z.shape[0], *z.shape[1:]), z.dtype) for z in zero_outs
        ]
        out_arrs = sharded(*concat_in, *concat_zeros)
        return [
            {
                n: np.asarray(out_arrs[i]).reshape(NC, *out_avals[i].shape)[c]
                for i, n in enumerate(out_names)
            }
            for c in range(NC)
        ]

    _cached["runner"] = (run, sharded, in_names, out_names, out_avals, zero_outs)
    return _cached["runner"]


def _make_in_maps(query, key, value, Wq, bq, Wk, bk, Wv, bv, Wo, bo):
    import ml_dtypes
    bf16 = ml_dtypes.bfloat16

    query = np.asarray(query, dtype=np.float32)
    key = np.asarray(key, dtype=np.float32)
    value = np.asarray(value, dtype=np.float32)
    Wq, Wk, Wv, Wo = (np.asarray(a, dtype=np.float32) for a in (Wq, Wk, Wv, Wo))
    bq = np.asarray(bq, dtype=np.float32)
    B = query.shape[0]
    ident = np.eye(128, dtype=bf16)

    xqT = [np.ascontiguousarray(query[b].T).astype(bf16) for b in range(B)]
    xkT = [np.ascontiguousarray(key[b].T).astype(bf16) for b in range(B)]
    xvT = [np.ascontiguousarray(value[b].T).astype(bf16) for b in range(B)]

    in_maps = []
    for c in range(NC):
        b, hg = divmod(c, NC // B)
        sl = slice(hg * M, (hg + 1) * M)
        in_maps.append(
            {
                "xqT": xqT[b],
                "xkT": xkT[b],
                "xvT": xvT[b],
                "wq": np.ascontiguousarray(Wq[:, sl]).astype(bf16),
                "wk": np.ascontiguousarray(Wk[:, sl]).astype(bf16),
                "wv": np.ascontiguousarray(Wv[:, sl]).astype(bf16),
                "wo": np.ascontiguousarray(Wo[sl, :]).astype(bf16),
                "bq": np.ascontiguousarray(bq[sl]),
                "ident": ident,
            }
        )
    return in_maps


def kernel(query, key, value, Wq, bq, Wk, bk, Wv, bv, Wo, bo):
    in_maps = _make_in_maps(query, key, value, Wq, bq, Wk, bk, Wv, bv, Wo, bo)
    run = _get_runner()[0]
    results = run(in_maps)

    B = np.asarray(query).shape[0]
    bo = np.asarray(bo, dtype=np.float32)
    bv = np.asarray(bv, dtype=np.float32)
    Wo_f = np.asarray(Wo, dtype=np.float32)
    base = bo + bv @ Wo_f  # bv contributes exactly bv @ Wo (sum of attn = 1)
    full = np.zeros((B, S, D), np.float32)
    for b in range(B):
        acc = np.zeros((S, D), np.float32)
        for g in range(NC // B):
            acc += results[b * (NC // B) + g]["out"]
        full[b] = acc + base[None, :]
    return full


# revision 19
# speedup vs baseline: 1.4267x; 1.0003x over previous
"""Multi-head attention (B=2, S=2048, D=1024, H=16) on 8 TRN2 NeuronCores.

Sharding: (batch, head-group) — core c handles batch c//4 and heads
[4*(c%4), 4*(c%4)+4). Each core projects its batch's tokens onto its 4 heads'
column-shards of Wq/Wk/Wv, runs attention for those heads, and multiplies by
its row-shard of Wo, producing a partial [S, D] output. The host sums the 4
partials per batch and adds (bo + bv @ Wo). bk is dropped entirely (a key
bias shifts every score of a query by the same constant, which softmax
cancels); bv contributes bv @ Wo to the output because attention weights sum
to one.

Device design notes:
  - All matmuls run in bf16 (1 cycle/row at any free size on the PE cost
    model); inputs and weights are cast to bf16 on the host, halving input
    DMA. PSUM accumulation stays f32.
  - Q/K are projected feature-major (out [m, s]; W stationary). V is
    projected token-major (x stationary, Wv moving) giving v in [keys, dk]
    layout directly — no V transpose pass. A constant-1 column appended to
    each per-(key-chunk, head) V block produces the softmax denominator
    inside the PV matmul.
  - Scores are computed transposed (S^T [key, query]) in [128, 2048] PSUM
    tiles (4 banks) so each Exp activation covers 2048 elements/partition,
    minimizing Act-engine fixed overhead. Act is the #2 engine (~121us).
  - PV is computed with queries on the OUTPUT partitions: out[q, dk+1] +=
    e_chunk^T-slice @ v_chunk. Free dim is 65 instead of 512, so PV costs
    half the baseline's PE cycles. Softmax normalization becomes a
    per-partition scalar multiply (DVE reciprocal of the denominator column
    + tensor_scalar_mul) — no partition broadcast needed.
  - ctx [q, m] is then PE-transposed per 128x128 block into ctx^T [m, q]
    for the Wo projection (contraction over m).
  - Emission order IS each engine's execution order. The schedule runs 8
    attention units (ih half x head) paced by the Act engine's exp stream;
    projections, PV of earlier units, transposes and Wo chunks are
    interleaved between score tiles via a static filler table.
  - PSUM: 2x1-bank pool (projections/transposes/Wo), 1x4-bank score tile,
    2x1-bank PV accumulators ([128, 4, 65] f32). Total exactly 8 banks.
"""

import numpy as np

S = 2048          # sequence length
D = 1024          # model dim
HPC = 4           # heads per core
DK = 64           # head dim
M = HPC * DK      # per-core projection width = 256
NC = 8            # cores
IW = 1024         # attention query width per ih-half
NDC = D // 128    # 8 contraction chunks
NMC = M // 128    # 2 m-chunks (head pairs)
NKB = S // 128    # 16 key chunks
EXPW = 1024       # exp tile width
NT = 16 * IW // EXPW  # qk/exp tiles per unit = 16

_cached = {}


def _build(debug=False):
    import concourse.bass as bass
    import concourse.bacc as bacc
    import concourse.tile as tile
    import concourse.mybir as mybir
    from contextlib import ExitStack

    f32 = mybir.dt.float32
    bf16 = mybir.dt.bfloat16
    AF = mybir.ActivationFunctionType

    nc = bacc.Bacc(
        "TRN2",
        target_bir_lowering=False,
        debug=False,
        enable_asserts=False,
        num_devices=NC,
    )

    # DRAM I/O (per-core shapes)
    xqT_d = nc.dram_tensor("xqT", [D, S], bf16, kind="ExternalInput").ap()
    xkT_d = nc.dram_tensor("xkT", [D, S], bf16, kind="ExternalInput").ap()
    xvT_d = nc.dram_tensor("xvT", [D, S], bf16, kind="ExternalInput").ap()
    wq_d = nc.dram_tensor("wq", [D, M], bf16, kind="ExternalInput").ap()
    wk_d = nc.dram_tensor("wk", [D, M], bf16, kind="ExternalInput").ap()
    wv_d = nc.dram_tensor("wv", [D, M], bf16, kind="ExternalInput").ap()
    wo_d = nc.dram_tensor("wo", [M, D], bf16, kind="ExternalInput").ap()
    bq_d = nc.dram_tensor("bq", [M], f32, kind="ExternalInput").ap()
    ident_d = nc.dram_tensor("ident", [128, 128], bf16, kind="ExternalInput").ap()
    out_d = nc.dram_tensor("out", [S, D], f32, kind="ExternalOutput").ap()

    with tile.TileContext(nc) as tc:
        with ExitStack() as st:
            wp = st.enter_context(tc.tile_pool(name="wp", bufs=1))
            xt = st.enter_context(tc.tile_pool(name="xt", bufs=14))
            qkt = st.enter_context(tc.tile_pool(name="qkt", bufs=1))
            vp = st.enter_context(tc.tile_pool(name="vp", bufs=1))
            ep = st.enter_context(tc.tile_pool(name="ep", bufs=40))
            stp = st.enter_context(tc.tile_pool(name="stp", bufs=1))
            ctp = st.enter_context(tc.tile_pool(name="ctp", bufs=1))
            invp = st.enter_context(tc.tile_pool(name="invp", bufs=4))
            ostp = st.enter_context(tc.tile_pool(name="ostp", bufs=4))
            # PSUM: exactly 8 banks
            psp = st.enter_context(tc.tile_pool(name="psp", bufs=2, space="PSUM"))
            qkp = st.enter_context(tc.tile_pool(name="qkp", bufs=2, space="PSUM"))
            pvp = st.enter_context(tc.tile_pool(name="pvp", bufs=2, space="PSUM"))

            wq_sb = wp.tile([128, NDC, M], bf16, tag="wq")
            wk_sb = wp.tile([128, NDC, M], bf16, tag="wk")
            wv_sb = wp.tile([128, NDC, M], bf16, tag="wv")
            wo_sb = wp.tile([128, NMC, D], bf16, tag="wo")
            bq_sb = wp.tile([128, NMC], f32, tag="bq")
            ident = wp.tile([128, 128], bf16, tag="ident")
            qT = [qkt.tile([128, S], bf16, tag=f"qT{m}", name=f"qT{m}")
                  for m in range(NMC)]
            kT = [qkt.tile([128, S], bf16, tag=f"kT{m}", name=f"kT{m}")
                  for m in range(NMC)]
            # v in [keys, head, dk+1] layout; col DK is the constant 1
            v_sb = vp.tile([128, NKB, HPC, DK + 1], bf16, tag="v")
            stage = [[stp.tile([128, M], bf16, tag=f"st{i}{q}", name=f"st{i}{q}")
                      for q in range(8)] for i in range(2)]
            ctx_t = [[ctp.tile([128, IW], bf16, tag=f"ct{i}{m}", name=f"ct{i}{m}")
                      for m in range(NMC)] for i in range(2)]

            nc.vector.memset(v_sb[:, :, :, DK:DK + 1], 1.0)

            w_r = lambda ap: ap.rearrange("(n p) m -> p n m", p=128)

            # ---- input DMA emission (order = SP.SEQ issue order) ----
            # x tensor-halves load as 4 chunked DMAs of [128, 2 dc, 1024]
            # each — few DMAs (HWDGE is a serial ~625ns/DMA resource) but
            # still streamable by the projections.
            xtiles = {}   # (tensor, dc-pair, sh) -> [128, 2, 1024] tile

            def load_x(tens, dram, sh):
                for dp in range(NDC // 2):
                    t = xt.tile([128, 2, 1024], bf16, tag="x",
                                name=f"x{tens}{dp}{sh}")
                    nc.sync.dma_start(
                        out=t,
                        in_=dram[dp * 256:(dp + 1) * 256,
                                 sh * 1024:(sh + 1) * 1024].rearrange(
                                     "(n p) s -> p n s", p=128))
                    xtiles[(tens, dp, sh)] = t

            def xap(tens, dc, sh):
                return xtiles[(tens, dc // 2, sh)][:, dc % 2, :]

            nc.sync.dma_start(out=wk_sb, in_=w_r(wk_d))
            nc.sync.dma_start(out=wq_sb, in_=w_r(wq_d))
            nc.sync.dma_start(out=bq_sb, in_=bq_d.rearrange("(n p) -> p n", p=128))
            nc.sync.dma_start(out=ident, in_=ident_d)
            load_x("k", xkT_d, 0)
            load_x("q", xqT_d, 0)
            load_x("k", xkT_d, 1)
            nc.sync.dma_start(out=wv_sb, in_=w_r(wv_d))
            load_x("v", xvT_d, 0)
            load_x("v", xvT_d, 1)
            load_x("q", xqT_d, 1)
            nc.sync.dma_start(out=wo_sb, in_=wo_d.rearrange("(g p) n -> p g n", p=128))

            # ---- emission helpers ----
            def proj_mk(tens, sh, mc, sc):
                """Project q/k chunk: out [m 128, s 512]; W stationary."""
                w_sb = wq_sb if tens == "q" else wk_sb
                ps = psp.tile([128, 512], f32, tag="ps", name="ps")
                for dc in range(NDC):
                    nc.tensor.matmul(
                        ps,
                        lhsT=w_sb[:, dc, mc * 128:(mc + 1) * 128],
                        rhs=xap(tens, dc, sh)[:, sc * 512:(sc + 1) * 512],
                        start=(dc == 0), stop=(dc == NDC - 1))
                dst = (qT if tens == "q" else kT)[mc][
                    :, sh * 1024 + sc * 512: sh * 1024 + (sc + 1) * 512]
                if tens == "q":
                    nc.vector.tensor_scalar_add(
                        out=dst, in0=ps, scalar1=bq_sb[:, mc:mc + 1])
                else:
                    nc.vector.tensor_copy(out=dst, in_=ps)

            def proj_v(kb):
                """Project v key-chunk kb: out [s 128, m 256]; x stationary."""
                sh, sc = divmod(kb, 8)
                ps = psp.tile([128, 512], f32, tag="ps", name="ps")
                for dc in range(NDC):
                    nc.tensor.matmul(
                        ps[:, 0:M],
                        lhsT=xap("v", dc, sh)[:, sc * 128:(sc + 1) * 128],
                        rhs=wv_sb[:, dc, :],
                        start=(dc == 0), stop=(dc == NDC - 1))
                for h in range(HPC):
                    nc.vector.tensor_copy(
                        out=v_sb[:, kb, h, 0:DK],
                        in_=ps[:, h * DK:(h + 1) * DK])

            e_tiles = {}
            SPLIT_U = 7  # last unit: tiles cover [4 kb x 256 queries] so each
                         # query-quarter closes out while later quarters exp

            def qk_tile(u, t):
                """Scores^T tile [keys 128, 1024] + exp -> e (bf16)."""
                ih, h = divmod(u, HPC)
                mc, off = divmod(h, 2)
                off *= DK
                qk = qkp.tile([128, EXPW], f32, tag="qk", name="qk")
                if u == SPLIT_U:
                    qq, kbp = divmod(t, 4)
                    blocks = [(4 * kbp + j, qq * 256, j * 256, 256)
                              for j in range(4)]
                else:
                    blocks = [(t, ha * 512, ha * 512, 512) for ha in range(2)]
                for kb, qoff, coff, w in blocks:
                    nc.tensor.matmul(
                        qk[:, coff:coff + w],
                        lhsT=kT[mc][off:off + DK, kb * 128:(kb + 1) * 128],
                        rhs=qT[mc][off:off + DK,
                                   ih * IW + qoff: ih * IW + qoff + w],
                        start=True, stop=True)
                e = ep.tile([128, EXPW], bf16, tag="e", name=f"e{u}_{t}")
                nc.scalar.activation(out=e, in_=qk, func=AF.Exp,
                                     scale=1.0 / np.sqrt(DK))
                e_tiles[(u, t)] = e

            def e_slice(u, kb, qc):
                """lhsT slice [keys 128, queries 128] of unit u's e tiles."""
                if u == SPLIT_U:
                    t = (qc // 2) * 4 + kb // 4
                    off = (kb % 4) * 256 + (qc % 2) * 128
                else:
                    t, off = kb, qc * 128
                return e_tiles[(u, t)][:, off:off + 128]

            pv_psum = {}

            def pv_qc(u, qc):
                """ctx[q 128, dk+1] for queries qc of unit u; accumulate all kb.
                Then normalize into stage (per-partition scalar multiply)."""
                ih, h = divmod(u, HPC)
                qg, q4 = divmod(qc, 4)
                if q4 == 0:
                    pv_psum[(u, qg)] = pvp.tile([128, 4, DK + 1], f32,
                                                tag="pv", name="pv")
                pv = pv_psum[(u, qg)]
                for kb in range(NKB):
                    nc.tensor.matmul(
                        pv[:, q4, :],
                        lhsT=e_slice(u, kb, qc),
                        rhs=v_sb[:, kb, h, :],
                        start=(kb == 0), stop=(kb == NKB - 1))
                inv = invp.tile([128, 1], f32, tag="inv", name="inv")
                nc.vector.reciprocal(out=inv, in_=pv[:, q4, DK:DK + 1])
                nc.vector.tensor_scalar_mul(
                    out=stage[ih][qc][:, h * DK:(h + 1) * DK],
                    in0=pv[:, q4, 0:DK], scalar1=inv)

            def t_qc(ih, qc, mc):
                """Transpose normalized ctx block [q 128, m 128] -> ctx_t."""
                ps = psp.tile([128, 512], bf16, tag="ps", name="tp")
                nc.tensor.transpose(
                    ps[:, 0:128],
                    in_=stage[ih][qc][:, mc * 128:(mc + 1) * 128],
                    identity=ident)
                nc.vector.tensor_copy(
                    out=ctx_t[ih][mc][:, qc * 128:(qc + 1) * 128],
                    in_=ps[:, 0:128])

            def wo_qc(ih, qc):
                """Output projection for query chunk qc of half ih."""
                for nh in range(2):
                    ps = psp.tile([128, 512], f32, tag="ps", name="wops")
                    for mc in range(NMC):
                        nc.tensor.matmul(
                            ps,
                            lhsT=ctx_t[ih][mc][:, qc * 128:(qc + 1) * 128],
                            rhs=wo_sb[:, mc, nh * 512:(nh + 1) * 512],
                            start=(mc == 0), stop=(mc == NMC - 1))
                    os_ = ostp.tile([128, 512], f32, tag="ost", name="ost")
                    nc.vector.tensor_copy(out=os_, in_=ps)
                    nc.sync.dma_start(
                        out=out_d[(ih * 8 + qc) * 128:(ih * 8 + qc + 1) * 128,
                                  nh * 512:(nh + 1) * 512],
                        in_=os_)

            # ---- static filler schedule ----
            PK = lambda tens, sh, mc, sc: (lambda: proj_mk(tens, sh, mc, sc))
            PJV = lambda kb: (lambda: proj_v(kb))
            PV = lambda u, qc: (lambda: pv_qc(u, qc))
            T = lambda ih, mc: (lambda: [t_qc(ih, qc, mc) for qc in range(8)])
            T1q = lambda qc: (lambda: t_qc(1, qc, 1))
            WO = lambda ih, qc: (lambda: wo_qc(ih, qc))

            FILL = {
                (0, 1): [PK("k", 0, 1, 0)], (0, 3): [PK("k", 0, 1, 1)],
                (0, 5): [PK("q", 0, 1, 0)],
                (0, 7): [PK("k", 1, 0, 0)], (0, 9): [PK("k", 1, 0, 1)],
                (0, 11): [PK("q", 0, 1, 1)],
                (0, 13): [PK("k", 1, 1, 0)],
                (1, 1): [PK("k", 1, 1, 1)],
                (1, 0): [PJV(0)], (1, 2): [PJV(1)], (1, 4): [PJV(2)],
                (1, 6): [PJV(3)], (1, 8): [PJV(4)], (1, 10): [PJV(5)],
                (1, 12): [PJV(6)], (1, 14): [PJV(7)],
                (2, 0): [PJV(8)], (2, 1): [PJV(9)], (2, 2): [PJV(10)],
                (2, 3): [PJV(11)], (2, 4): [PJV(12)], (2, 5): [PJV(13)],
                (2, 6): [PJV(14)], (2, 7): [PJV(15)],
                (2, 8): [PV(0, 0)], (2, 9): [PV(0, 1)], (2, 10): [PV(0, 2)],
                (2, 11): [PV(0, 3)], (2, 12): [PV(0, 4)], (2, 13): [PV(0, 5)],
                (2, 14): [PV(0, 6)], (2, 15): [PV(0, 7)],
                (3, 0): [PV(1, 0)], (3, 1): [PV(1, 1)], (3, 2): [PV(1, 2)],
                (3, 3): [PV(1, 3)], (3, 4): [PV(1, 4)], (3, 5): [PV(1, 5)],
                (3, 6): [PV(1, 6)], (3, 7): [PV(1, 7)],
                (3, 8): [PK("q", 1, 0, 0)], (3, 10): [PK("q", 1, 0, 1)],
                (3, 12): [PK("q", 1, 1, 0)], (3, 14): [PK("q", 1, 1, 1)],
                (4, 0): [PV(2, 0)], (4, 1): [PV(2, 1)], (4, 2): [PV(2, 2)],
                (4, 3): [PV(2, 3)], (4, 4): [PV(2, 4)], (4, 5): [PV(2, 5)],
                (4, 6): [PV(2, 6)], (4, 7): [PV(2, 7)],
                (4, 8): [PV(3, 0)], (4, 9): [PV(3, 1)], (4, 10): [PV(3, 2)],
                (4, 11): [PV(3, 3)], (4, 12): [PV(3, 4)], (4, 13): [PV(3, 5)],
                (4, 14): [PV(3, 6)], (4, 15): [PV(3, 7)],
                (5, 0): [T(0, 0)], (5, 1): [T(0, 1)],
                (5, 2): [WO(0, 0)], (5, 4): [WO(0, 1)], (5, 6): [WO(0, 2)],
                (5, 8): [WO(0, 3)], (5, 10): [WO(0, 4)], (5, 12): [WO(0, 5)],
                (5, 14): [WO(0, 6)], (5, 15): [WO(0, 7)],
                (6, 0): [PV(4, 0)], (6, 1): [PV(4, 1)], (6, 2): [PV(4, 2)],
                (6, 3): [PV(4, 3)], (6, 4): [PV(4, 4)], (6, 5): [PV(4, 5)],
                (6, 6): [PV(4, 6)], (6, 7): [PV(4, 7)],
                (6, 8): [PV(5, 0)], (6, 9): [PV(5, 1)], (6, 10): [PV(5, 2)],
                (6, 11): [PV(5, 3)], (6, 12): [PV(5, 4)], (6, 13): [PV(5, 5)],
                (6, 14): [PV(5, 6)], (6, 15): [PV(5, 7)],
                (7, 0): [T(1, 0)],
                (7, 1): [PV(6, 0)], (7, 2): [PV(6, 1)], (7, 3): [PV(6, 2)],
                (7, 4): [PV(6, 3)],
                (7, 5): [PV(7, 0)],
                (7, 6): [PV(7, 1), T1q(0)],
                (7, 7): [PV(6, 4), WO(1, 0)],
                (7, 8): [PV(6, 5), T1q(1)],
                (7, 9): [PV(6, 6), WO(1, 1)],
                (7, 10): [PV(6, 7), PV(7, 2)],
                (7, 11): [PV(7, 3), T1q(2)],
                (7, 12): [WO(1, 2), T1q(3)],
                (7, 13): [WO(1, 3), PV(7, 4)],
                (7, 14): [PV(7, 5), T1q(4)],
                (7, 15): [WO(1, 4)],
            }

            def proj_mk_pair(tens, sh, mc):
                """Both 512-col groups of a projection, dc-major interleaved
                so the last matmuls land right as the final x chunk arrives."""
                w_sb = wq_sb if tens == "q" else wk_sb
                pss = [psp.tile([128, 512], f32, tag="ps", name="ps")
                       for _ in range(2)]
                for dc in range(NDC):
                    for sc in range(2):
                        nc.tensor.matmul(
                            pss[sc],
                            lhsT=w_sb[:, dc, mc * 128:(mc + 1) * 128],
                            rhs=xap(tens, dc, sh)[:, sc * 512:(sc + 1) * 512],
                            start=(dc == 0), stop=(dc == NDC - 1))
                for sc in range(2):
                    dst = (qT if tens == "q" else kT)[mc][
                        :, sh * 1024 + sc * 512: sh * 1024 + (sc + 1) * 512]
                    if tens == "q":
                        nc.vector.tensor_scalar_add(
                            out=dst, in0=pss[sc], scalar1=bq_sb[:, mc:mc + 1])
                    else:
                        nc.vector.tensor_copy(out=dst, in_=pss[sc])

            # ---- main pipeline ----
            # prologue: first projections (DMA-gated)
            proj_mk_pair("k", 0, 0)
            proj_mk_pair("q", 0, 0)
            # 8 attention units paced by the exp stream
            for u in range(8):
                for t in range(NT):
                    qk_tile(u, t)
                    for fn in FILL.get((u, t), []):
                        fn()
            # tail: close out the last unit's final query-quarters
            pv_qc(7, 6)
            t_qc(1, 5, 1)
            wo_qc(1, 5)
            pv_qc(7, 7)
            t_qc(1, 6, 1)
            wo_qc(1, 6)
            t_qc(1, 7, 1)
            wo_qc(1, 7)

    nc.compile()
    return nc


def _get_nc(debug=False):
    key = ("nc", debug)
    if key not in _cached:
        _cached[key] = _build(debug)
    return _cached[key]


def _get_runner():
    """Build (once) a jitted 8-core SPMD executable mirroring
    bass2jax.run_bass_via_pjrt, reusable across calls for benchmarking."""
    if "runner" in _cached:
        return _cached["runner"]
    import jax
    import jax.numpy as jnp
    from jax.experimental.shard_map import shard_map
    from jax.sharding import Mesh, PartitionSpec
    import concourse.mybir as mybir
    from concourse import bass2jax

    bass2jax.install_neuronx_cc_hook()
    nc = _get_nc()
    assert nc.dbg_addr is None
    partition_name = nc.partition_id_tensor.name if nc.partition_id_tensor else None

    in_names, out_names, out_avals, zero_outs = [], [], [], []
    for alloc in nc.m.functions[0].allocations:
        if not isinstance(alloc, mybir.MemoryLocationSet):
            continue
        name = alloc.memorylocations[0].name
        if alloc.kind == "ExternalInput":
            if name != partition_name:
                in_names.append(name)
        elif alloc.kind == "ExternalOutput":
            out_names.append(name)
            shape = tuple(alloc.tensor_shape)
            dtype = mybir.dt.np(alloc.dtype)
            out_avals.append(jax.core.ShapedArray(shape, dtype))
            zero_outs.append(np.zeros(shape, dtype))
    n_params = len(in_names)
    all_in_names = in_names + out_names
    if partition_name is not None:
        all_in_names = all_in_names + [partition_name]
    donate = tuple(range(n_params, n_params + len(out_names)))

    def _body(*args):
        operands = list(args)
        if partition_name is not None:
            operands.append(bass2jax.partition_id_tensor())
        outs = bass2jax._bass_exec_p.bind(
            *operands,
            out_avals=tuple(out_avals),
            in_names=tuple(all_in_names),
            out_names=tuple(out_names),
            lowering_input_output_aliases=(),
            sim_require_finite=True,
            sim_require_nnan=True,
            nc=nc,
        )
        return tuple(outs)

    devices = jax.devices()[:NC]
    mesh = Mesh(np.asarray(devices), ("core",))
    nin = n_params + len(out_names)
    sharded = jax.jit(
        shard_map(
            _body,
            mesh=mesh,
            in_specs=(PartitionSpec("core"),) * nin,
            out_specs=(PartitionSpec("core"),) * len(out_names),
            check_rep=False,
        ),
        donate_argnums=donate,
        keep_unused=True,
    )

    def run(in_maps):
        concat_in = [
            np.concatenate([np.asarray(in_maps[c][n]) for c in range(NC)], axis=0)
            for n in in_names
        ]
        concat_zeros = [
            np.zeros((NC * z.shape[0], *z.shape[1:]), z.dtype) for z in zero_outs
        ]
        out_arrs = sharded(*concat_in, *concat_zeros)
        return [
            {
                n: np.asarray(out_arrs[i]).reshape(NC, *out_avals[i].shape)[c]
                for i, n in enumerate(out_names)
            }
            for c in range(NC)
        ]

    _cached["runner"] = (run, sharded, in_names, out_names, out_avals, zero_outs)
    return _cached["runner"]


def _make_in_maps(query, key, value, Wq, bq, Wk, bk, Wv, bv, Wo, bo):
    import ml_dtypes
    bf16 = ml_dtypes.bfloat16

    query = np.asarray(query, dtype=np.float32)
    key = np.asarray(key, dtype=np.float32)
    value = np.asarray(value, dtype=np.float32)
    Wq, Wk, Wv, Wo = (np.asarray(a, dtype=np.float32) for a in (Wq, Wk, Wv, Wo))
    bq = np.asarray(bq, dtype=np.float32)
    B = query.shape[0]
    ident = np.eye(128, dtype=bf16)

    xqT = [np.ascontiguousarray(query[b].T).astype(bf16) for b in range(B)]
    xkT = [np.ascontiguousarray(key[b].T).astype(bf16) for b in range(B)]
    xvT = [np.ascontiguousarray(value[b].T).astype(bf16) for b in range(B)]

    in_maps = []
    for c in range(NC):
        b, hg = divmod(c, NC // B)
        sl = slice(hg * M, (hg + 1) * M)
        in_maps.append(
            {
                "xqT": xqT[b],
                "xkT": xkT[b],
                "xvT": xvT[b],
                "wq": np.ascontiguousarray(Wq[:, sl]).astype(bf16),
                "wk": np.ascontiguousarray(Wk[:, sl]).astype(bf16),
                "wv": np.ascontiguousarray(Wv[:, sl]).astype(bf16),
                "wo": np.ascontiguousarray(Wo[sl, :]).astype(bf16),
                "bq": np.ascontiguousarray(bq[sl]),
                "ident": ident,
            }
        )
    return in_maps


def kernel(query, key, value, Wq, bq, Wk, bk, Wv, bv, Wo, bo):
    in_maps = _make_in_maps(query, key, value, Wq, bq, Wk, bk, Wv, bv, Wo, bo)
    run = _get_runner()[0]
    results = run(in_maps)

    B = np.asarray(query).shape[0]
    bo = np.asarray(bo, dtype=np.float32)
    bv = np.asarray(bv, dtype=np.float32)
    Wo_f = np.asarray(Wo, dtype=np.float32)
    base = bo + bv @ Wo_f  # bv contributes exactly bv @ Wo (sum of attn = 1)
    full = np.zeros((B, S, D), np.float32)
    for b in range(B):
        acc = np.zeros((S, D), np.float32)
        for g in range(NC // B):
            acc += results[b * (NC // B) + g]["out"]
        full[b] = acc + base[None, :]
    return full
